# revision 18
# baseline (speedup 1.0000x reference)
"""Bass/Tile kernel for nn_CMCD (annealed Langevin sampler with SVGD repulsion).

SPMD over 8 cores, data-parallel over the particle batch (64 rows/core).

Structure (v2):
- Host precomputes all input-only transforms: time-embedding table, betas,
  weight layouts/casts, noise prescale, and the step-0 particle tiles
  (so step 0 needs no collective).
- A tiny warm-up AllGather fires at t~0 to absorb collective mesh-init /
  core-start skew while weights stream in.
- Per step s>=1: AllGather of x_s posted at the end of step s-1; the
  score net + mixture-gradient run in its shadow; repulsion from the
  gathered particles; fused update.
- Activation-table discipline: steady-state act functions are only
  {Exp, Gelu, Identity, Square} ordered as [exp-block][gelu-block] per
  step -> 2 table loads/step.
- Bandwidth (SVGD median heuristic) replaced by a calibrated sqrt-free
  estimator computed from mean/var of d2 over a 128x512 subsample, one
  step stale (h_s = h(x_{s-1})); step 0/1 bandwidth comes from the host.
  Validated end-to-end at rel err ~2e-5 vs the jax reference.
"""
import numpy as np
from contextlib import ExitStack

import concourse.bass as bass
import concourse.bacc as bacc
import concourse.tile as tile
from concourse import mybir
from concourse.masks import make_identity

D, C, NB, NH, M = 64, 512, 8, 3, 8
B = 512
NCORES = 8
BL = B // NCORES  # 64
KB = C // 128     # 4 channel blocks
LOGN = float(np.log(B))
RSUB = 128 * B    # subsample count for the bandwidth (rows 0..127)
AGW = BL * D + BL  # flat AllGather payload words per core
NJUNK = 400
EPS_A = 2.0        # total d2 shift (bf16-safety); corrected exactly
F32 = mybir.dt.float32
BF16 = mybir.dt.bfloat16
AF = mybir.ActivationFunctionType
ALU = mybir.AluOpType
GELU = AF.Gelu_apprx_tanh


def build_nc(compile=True):
    nc = bacc.Bacc("TRN2", target_bir_lowering=False, debug=False,
                   num_devices=NCORES)

    t = {}
    def din(name, shape, dtype):
        t[name] = nc.dram_tensor(name, shape, dtype, kind="ExternalInput")

    # ---- per-core state inputs ----
    din("x0_loc", [BL, D], F32)
    din("xT0_loc", [D, BL], F32)
    din("xT0_locN2", [D, BL], BF16)
    din("x2locn2_0", [1, BL], BF16)
    din("xall0", [D, B], BF16)        # -2 * x0^T, all particles
    din("x2rowN2_0", [1, B], BF16)    # -2*(|x0_j|^2 + 1), all particles
    din("noises_s", [BL, NB, D], F32)  # pre-scaled by sqrt(2 dt)
    # ---- weights / tables (host-prepped) ----
    din("inWs_bf", [D, C], BF16)       # -0.5 * in_W
    din("te_bf", [1, NB * C], BF16)    # te_s + in_b, flat row
    din("hW_bf", [128, NH * KB * C], BF16)
    din("hb_bf", [1, NH * C], BF16)
    din("outWs_bf", [128, KB * D], BF16)  # dt * out_W
    din("outbs_bf", [1, D], BF16)         # dt * out_b
    din("means", [M, D], F32)
    din("meansT", [D, M], F32)
    din("negmu2", [1, M], F32)
    din("dtb8", [M, NB], F32)          # col s = -dt*beta_s
    din("row4", [1, 4], F32)           # [.5*logn, A*logn, -.05*dt*logn, .1*dt*logn]
    din("bcast0_row", [1, 4], F32)     # row4 / hL(x0)
    din("omd_col", [128, 1], F32)      # 1 - dt

    traj_d = nc.dram_tensor("traj", [NB, BL, D], F32, kind="ExternalOutput")
    t["traj_d"] = traj_d

    # collective bounce buffers: steps 1..NB-1, plus a warm-up dummy
    t["agin"] = [None] + [nc.dram_tensor(f"agin{s}", [AGW], BF16)
                          for s in range(1, NB)]
    t["agout"] = [None] + [nc.dram_tensor(f"agout{s}", [NCORES, AGW], BF16,
                                          addr_space="Shared")
                           for s in range(1, NB)]
    t["dd_in"] = nc.dram_tensor("dd_in", [64], BF16)
    t["dd_out"] = nc.dram_tensor("dd_out", [NCORES, 64], BF16,
                                 addr_space="Shared")

    with tile.TileContext(nc) as tc, ExitStack() as ctx:
        _body(ctx, tc, nc, t)
    if compile:
        nc.compile()
    return nc


def _body(ctx, tc, nc, t):
    traj_d, agin, agout = t["traj_d"], t["agin"], t["agout"]

    const = ctx.enter_context(tc.tile_pool(name="const", bufs=1))
    wpool = ctx.enter_context(tc.tile_pool(name="wpool", bufs=1))
    sb2 = ctx.enter_context(tc.tile_pool(name="sb2", bufs=2))
    sb3 = ctx.enter_context(tc.tile_pool(name="sb3", bufs=3))
    scratch = ctx.enter_context(tc.tile_pool(name="scratch", bufs=2))
    ps_small = ctx.enter_context(tc.tile_pool(name="ps_small", bufs=1, space="PSUM"))
    ps_junk = ctx.enter_context(tc.tile_pool(name="ps_junk", bufs=1, space="PSUM"))
    ps_d2f = ctx.enter_context(tc.tile_pool(name="ps_d2f", bufs=1, space="PSUM"))
    ps_d2l = ctx.enter_context(tc.tile_pool(name="ps_d2l", bufs=1, space="PSUM"))
    ps_u = ctx.enter_context(tc.tile_pool(name="ps_u", bufs=1, space="PSUM"))
    ps_cmp = ctx.enter_context(tc.tile_pool(name="ps_cmp", bufs=1, space="PSUM"))
    ps_net = ctx.enter_context(tc.tile_pool(name="ps_net", bufs=2, space="PSUM"))

    # ---------------- warm-up collective: very first instruction ----------------
    nc.gpsimd.collective_compute(
        "AllGather", ALU.bypass, replica_groups=[list(range(NCORES))],
        ins=[t["dd_in"].ap().opt()], outs=[t["dd_out"].ap().opt()])

    # ---------------- input DMAs (3 queues, ordered by first use) ----------------
    # queue A (sync): step-0 particle tiles + noises
    x0_loc = wpool.tile([BL, D], F32)
    nc.sync.dma_start(out=x0_loc, in_=t["x0_loc"][:, :])
    xT0_loc = wpool.tile([D, BL], F32)
    nc.sync.dma_start(out=xT0_loc, in_=t["xT0_loc"][:, :])
    xT0_locN2 = wpool.tile([D, BL], BF16)
    nc.sync.dma_start(out=xT0_locN2, in_=t["xT0_locN2"][:, :])
    x2locn2_0 = wpool.tile([1, BL], BF16)
    nc.sync.dma_start(out=x2locn2_0, in_=t["x2locn2_0"][:, :])
    xall0 = wpool.tile([D, NCORES, BL], BF16)
    nc.sync.dma_start(out=xall0, in_=t["xall0"].ap().rearrange(
        "d (c b) -> d c b", c=NCORES))
    x2rowN2_0 = wpool.tile([1, B], BF16)
    nc.sync.dma_start(out=x2rowN2_0, in_=t["x2rowN2_0"][:, :])
    noise_sb = wpool.tile([BL, NB, D], F32)
    nc.sync.dma_start(out=noise_sb, in_=t["noises_s"][:, :, :])
    # queue A continues: second half of hW
    hW_sb = wpool.tile([128, NH, KB, C], BF16)
    # queue B (scalar): small weights in use order
    inWs_bf = wpool.tile([D, C], BF16)
    nc.scalar.dma_start(out=inWs_bf, in_=t["inWs_bf"][:, :])
    te_bf = wpool.tile([1, NB * C], BF16)
    nc.scalar.dma_start(out=te_bf[0:1, 0:NB * C // 2],
                        in_=t["te_bf"][0:1, 0:NB * C // 2])
    nc.sync.dma_start(out=te_bf[0:1, NB * C // 2:],
                      in_=t["te_bf"][0:1, NB * C // 2:])
    meansT_sb = wpool.tile([D, M], F32)
    nc.scalar.dma_start(out=meansT_sb, in_=t["meansT"][:, :])
    negmu2_row = wpool.tile([1, M], F32)
    nc.scalar.dma_start(out=negmu2_row, in_=t["negmu2"][:, :])
    means_sb = wpool.tile([M, D], F32)
    nc.scalar.dma_start(out=means_sb, in_=t["means"][:, :])
    dtb8_sb = wpool.tile([M, NB], F32)
    nc.scalar.dma_start(out=dtb8_sb, in_=t["dtb8"][:, :])
    row4_sb = wpool.tile([1, 4], F32)
    nc.scalar.dma_start(out=row4_sb, in_=t["row4"][:, :])
    bcast0_row = wpool.tile([1, 4], F32)
    nc.scalar.dma_start(out=bcast0_row, in_=t["bcast0_row"][:, :])
    omd_col = wpool.tile([128, 1], F32)
    nc.scalar.dma_start(out=omd_col, in_=t["omd_col"][:, :])
    hb_sb = wpool.tile([1, NH * C], BF16)
    nc.scalar.dma_start(out=hb_sb, in_=t["hb_bf"][:, :])
    outWs_sb = wpool.tile([128, KB, D], BF16)
    nc.scalar.dma_start(out=outWs_sb, in_=t["outWs_bf"].ap().rearrange(
        "p (k d) -> p k d", k=KB))
    outbs_row = wpool.tile([1, D], BF16)
    nc.scalar.dma_start(out=outbs_row, in_=t["outbs_bf"][:, :])
    # hW split across queues A and B (~750KB each)
    hWr = t["hW_bf"].ap().rearrange("p (l k c) -> p l k c", l=NH, k=KB)
    nc.scalar.dma_start(out=hW_sb[:, 0:2, :, :], in_=hWr[:, 0:2, :, :])
    nc.sync.dma_start(out=hW_sb[:, 2:NH, :, :], in_=hWr[:, 2:NH, :, :])

    # ---------------- constants ----------------
    ident = const.tile([128, 128], F32)
    make_identity(nc, ident)
    ident_bf = const.tile([128, 128], BF16)
    nc.vector.tensor_copy(ident_bf, ident)
    ones_col = const.tile([128, 1], F32)
    nc.vector.memset(ones_col, 1.0)
    ones_row = const.tile([1, 128], F32)
    nc.vector.memset(ones_row, 1.0)
    ones_row_bf = const.tile([1, 128], BF16)
    nc.vector.memset(ones_row_bf, 1.0)
    ones_col_bf = const.tile([128, 1], BF16)
    nc.vector.memset(ones_col_bf, 1.0)

    # broadcast bcast0_row -> [128, 4] (used by steps 0 and 1)
    bc0_ps = ps_small.tile([128, 4], F32, tag="sm", name="bc0_ps")
    nc.tensor.matmul(bc0_ps, lhsT=ones_row, rhs=bcast0_row, start=True, stop=True)
    bc0 = const.tile([128, 4], F32)
    nc.vector.tensor_copy(bc0, bc0_ps)

    # ---------------- per-step state handles ----------------
    x_loc = x0_loc
    xT_loc = xT0_loc
    xT_locN2 = xT0_locN2
    x2locn2 = x2locn2_0
    bc_next = bc0  # bandwidth broadcast for the *next* issued step

    for s in range(NB):
        bc = bc_next
        # ---- gathered particle tiles ----
        if s == 0:
            xall = xall0
            x2rowN2 = x2rowN2_0
        else:
            xall = sb2.tile([D, NCORES, BL], BF16, tag="xall", name=f"xall{s}")
            for half, eng in ((0, nc.sync), (1, nc.scalar)):
                eng.dma_start(
                    out=xall[:, half * 4:(half + 1) * 4, :],
                    in_=bass.AP(tensor=agout[s].ap().tensor,
                                offset=half * 4 * AGW,
                                ap=[[BL, D], [AGW, 4], [1, BL]]))
            x2rowN2 = sb3.tile([1, B], BF16, tag="x2rowN2", name=f"x2r{s}")
            nc.sync.dma_start(
                out=x2rowN2.rearrange("o (c b) -> o c b", c=NCORES),
                in_=bass.AP(tensor=agout[s].ap().tensor, offset=BL * D,
                            ap=[[0, 1], [AGW, NCORES], [1, BL]]))

        # ================= TENSOR (+ matching act/vector) =================
        # ---- mixture-component logits (feeds softmax exp, first act op) ----
        comp_ps = ps_cmp.tile([BL, M], F32, tag="cmp", name=f"comp{s}")
        nc.tensor.matmul(comp_ps, lhsT=xT_loc, rhs=meansT_sb, start=True, stop=False)
        nc.tensor.matmul(comp_ps, lhsT=ones_row[0:1, 0:BL], rhs=negmu2_row,
                         start=False, stop=True)
        negmax = sb3.tile([BL, 1], F32, tag="negmax", name=f"negmax{s}")
        nc.vector.tensor_reduce(negmax, comp_ps, axis=mybir.AxisListType.X,
                                op=ALU.max, negate=True)
        # softmax exps via Tanh (resident in the gelu table):
        # e^-w = (1-tanh(w/2))/(1+tanh(w/2)), w = -(comp+negmax) >= 0
        halfmax = sb3.tile([BL, 1], F32, tag="halfmax", name=f"halfmax{s}")
        nc.vector.tensor_scalar(halfmax, negmax, -0.5, None, ALU.mult)
        tsm = sb3.tile([BL, M], F32, tag="tsm", name=f"tsm{s}")
        nc.scalar.activation(tsm, comp_ps, AF.Tanh, bias=halfmax, scale=-0.5)
        numt = sb3.tile([BL, M], F32, tag="numt", name=f"numt{s}")
        nc.vector.tensor_scalar(numt, tsm, -1.0, 1.0, ALU.mult, ALU.add)
        dent = sb3.tile([BL, M], F32, tag="dent", name=f"dent{s}")
        nc.vector.tensor_scalar(dent, tsm, 1.0, 1.0, ALU.mult, ALU.add)
        rden = sb3.tile([BL, M], F32, tag="rden", name=f"rden{s}")
        nc.vector.reciprocal(rden, dent)
        w_un = sb3.tile([BL, M], F32, tag="w_un", name=f"w_un{s}")
        nc.vector.tensor_tensor(w_un, numt, rden, ALU.mult)
        sumexp = sb3.tile([BL, 1], F32, tag="sumexp", name=f"sumexp{s}")
        nc.vector.tensor_reduce(sumexp, w_un, axis=mybir.AxisListType.X,
                                op=ALU.add)
        rcp = sb3.tile([BL, 1], F32, tag="rcp", name=f"rcp{s}")
        nc.vector.reciprocal(rcp, sumexp)
        w_n = sb3.tile([BL, M], F32, tag="w_n", name=f"w_n{s}")
        nc.vector.tensor_scalar(w_n, w_un, rcp, None, ALU.mult)

        # ---- score net layer 1: h^T = (x @ in_W)^T + te (gelu) ----
        h_ps = ps_net.tile([128, KB, BL], F32, tag="h_ps", bufs=1, name=f"h_ps{s}")
        for ko in range(KB):
            nc.tensor.matmul(h_ps[:, ko, :],
                             lhsT=inWs_bf[:, 128 * ko:128 * (ko + 1)],
                             rhs=xT_locN2, start=True, stop=False)
            nc.tensor.matmul(h_ps[:, ko, :],
                             lhsT=te_bf[0:1, s * C + 128 * ko:
                                        s * C + 128 * (ko + 1)],
                             rhs=ones_row_bf[0:1, 0:BL], start=False, stop=True)
        h_sb = sb2.tile([128, KB, BL], BF16, tag="h0", name=f"h0_{s}")
        # act #2: Gelu (table load; net has slack vs the collective path)
        nc.scalar.activation(h_sb.rearrange("p k b -> p (k b)"),
                             h_ps.rearrange("p k b -> p (k b)"), GELU)

        # ---- softmax tail on tensor (w^T, scaled by -dt*beta_s) ----
        wT_ps = ps_small.tile([M, BL], F32, tag="sm", name=f"wT{s}")
        nc.tensor.transpose(wT_ps, w_n, ident[0:BL, 0:BL])
        wTs_sb = sb3.tile([M, BL], F32, tag="wTs", name=f"wTs{s}")
        nc.vector.tensor_scalar(wTs_sb, wT_ps, dtb8_sb[0:M, s:s + 1], None, ALU.mult)

        # ---- hidden layers ----
        for l in range(NH):
            hu_ps = ps_net.tile([BL, C], F32, tag="hu", bufs=1, name=f"hu{s}_{l}")
            for ki in range(KB):
                nc.tensor.matmul(hu_ps, lhsT=h_sb[:, ki, :], rhs=hW_sb[:, l, ki, :],
                                 start=(ki == 0), stop=False)
            nc.tensor.matmul(hu_ps, lhsT=ones_row_bf[0:1, 0:BL],
                             rhs=hb_sb[0:1, l * C:(l + 1) * C],
                             start=False, stop=True)
            hu_sb = sb2.tile([BL, C], BF16, tag="hu_sb", name=f"hu_sb{s}_{l}")
            nc.vector.tensor_copy(hu_sb, hu_ps)
            tps = ps_net.tile([128, KB, BL], BF16, tag="h_ps", bufs=1,
                              name=f"tps{s}_{l}")
            for k in range(KB):
                nc.tensor.transpose(tps[:, k, :], hu_sb[:, 128 * k:128 * (k + 1)],
                                    ident_bf[0:BL, 0:BL])
            hn_sb = sb2.tile([128, KB, BL], BF16, tag=f"h{l + 1}",
                             name=f"hn_sb{s}_{l}")
            nc.scalar.activation(hn_sb.rearrange("p k b -> p (k b)"),
                                 tps.rearrange("p k b -> p (k b)"), GELU)
            h_sb = hn_sb

        # ---- pairwise d2, local columns: psum = -2*(d2+A) ----
        d2l_ps = ps_d2l.tile([128, KB, BL], F32, tag="d2l", name=f"d2l{s}")
        for k in range(KB):
            nc.tensor.matmul(d2l_ps[:, k, :], lhsT=xall[:, 2 * k:2 * k + 2, :],
                             rhs=xT_locN2, start=True, stop=False)
            nc.tensor.matmul(d2l_ps[:, k, :], lhsT=ones_row_bf[0:1, 0:128],
                             rhs=x2locn2, start=False, stop=False)
            nc.tensor.matmul(d2l_ps[:, k, :],
                             lhsT=x2rowN2[0:1, 128 * k:128 * (k + 1)],
                             rhs=ones_row_bf[0:1, 0:BL], start=False, stop=True)

        # ---- repulsion kernel: kt = exp(-d2/h) ----
        kt_sb = sb2.tile([128, KB, BL], BF16, tag="kt", name=f"kt{s}")
        # act: Exp (table load #2; hides under the collective/d2l window)
        nc.scalar.activation(kt_sb.rearrange("p k b -> p (k b)"),
                             d2l_ps.rearrange("p k b -> p (k b)"), AF.Exp,
                             bias=bc[:, 1:2], scale=bc[:, 0:1])

        # ---- x rows scaled by c_h: xfe = c_h * x  (from -2x^T blocks) ----
        xft_ps = ps_net.tile([128, KB, BL], BF16, tag="h_ps", bufs=1,
                             name=f"xft{s}")
        for k in range(KB):
            nc.tensor.transpose(xft_ps[:, k, :], xall[:, 2 * k:2 * k + 2, :],
                                ident_bf[0:D, 0:D])
        xfe = sb2.tile([128, KB, BL], BF16, tag="xfe", name=f"xfe{s}")
        nc.vector.tensor_scalar(xfe.rearrange("p k b -> p (k b)"),
                                xft_ps.rearrange("p k b -> p (k b)"),
                                bc[:, 2:3], None, ALU.mult)

        # ---- U = dt*score + dt*out_b - dt*beta*(w@means) ----
        ur_ps = ps_u.tile([BL, 2, D], F32, tag="u", name=f"ur{s}")
        u_ps = ur_ps[:, 0, :]
        for ki in range(KB):
            nc.tensor.matmul(u_ps, lhsT=h_sb[:, ki, :], rhs=outWs_sb[:, ki, :],
                             start=(ki == 0), stop=False)
        nc.tensor.matmul(u_ps, lhsT=ones_row_bf[0:1, 0:BL], rhs=outbs_row,
                         start=False, stop=False)
        nc.tensor.matmul(u_ps, lhsT=wTs_sb, rhs=means_sb, start=False, stop=True)

        # ---- kxr = c_h * K@x ; rch = c_h * r ----
        kxr_ps = ur_ps[:, 1, :]
        for k in range(KB):
            nc.tensor.matmul(kxr_ps, lhsT=kt_sb[:, k, :], rhs=xfe[:, k, :],
                             start=(k == 0), stop=(k == KB - 1))
        chcol_bf = sb3.tile([128, 1], BF16, tag="chcol", name=f"chcol{s}")
        nc.vector.tensor_copy(chcol_bf, bc[:, 3:4])
        rch_ps = ps_small.tile([BL, 1], F32, tag="sm", name=f"rch{s}")
        for k in range(KB):
            nc.tensor.matmul(rch_ps, lhsT=kt_sb[:, k, :], rhs=chcol_bf,
                             start=(k == 0), stop=(k == KB - 1))

        # ---- update: new = x*(1-dt+c_h*r) + noise - U - c_h*K@x ----
        alpha = sb3.tile([BL, 1], F32, tag="alpha", name=f"alpha{s}")
        nc.vector.tensor_tensor(alpha, rch_ps, omd_col[0:BL, 0:1], ALU.add)
        t1 = sb3.tile([BL, D], F32, tag="t1", name=f"t1_{s}")
        nc.vector.tensor_scalar(t1, x_loc, alpha, None, ALU.mult)
        t2 = sb3.tile([BL, D], F32, tag="t2", name=f"t2_{s}")
        nc.vector.tensor_tensor(t2, t1, noise_sb[:, s, :], ALU.add)
        t3 = sb3.tile([BL, D], F32, tag="t3", name=f"t3_{s}")
        nc.vector.tensor_tensor(t3, t2, u_ps, ALU.subtract)
        new_x = sb2.tile([BL, D], F32, tag="x_loc", name=f"x{s + 1}")
        nc.vector.tensor_tensor(new_x, t3, kxr_ps, ALU.subtract)
        nc.scalar.dma_start(out=traj_d[s], in_=new_x)

        # ---- stage + post AllGather for step s+1 (one DMA: [-2x^T | x2]) ----
        if s + 1 < NB:
            nxT_ps = ps_small.tile([D, BL], F32, tag="sm", name=f"nxT{s + 1}")
            nc.tensor.transpose(nxT_ps, new_x, ident[0:BL, 0:BL])
            stg = sb2.tile([D + 1, BL], BF16, tag="stg", name=f"stg{s + 1}")
            nc.vector.tensor_scalar(stg[0:D, :], nxT_ps, -2.0, None, ALU.mult)
            nxT_loc = sb2.tile([D, BL], F32, tag="xT_loc", name=f"xT{s + 1}")
            nc.vector.tensor_copy(nxT_loc, nxT_ps)
            sqnT = scratch.tile([D, BL], F32, tag="sqnT", name=f"sqnT{s + 1}")
            nc.vector.tensor_tensor(sqnT, nxT_ps, nxT_loc, ALU.mult)
            x2l_ps = ps_small.tile([1, BL], F32, tag="sm", name=f"x2l{s + 1}")
            nc.tensor.matmul(x2l_ps, lhsT=ones_col[0:D, 0:1], rhs=sqnT,
                             start=True, stop=True)
            nc.vector.tensor_scalar(stg[D:D + 1, :], x2l_ps, -2.0, -2.0,
                                    ALU.mult, ALU.add)
            nc.sync.dma_start(
                out=agin[s + 1].ap()[0:AGW].rearrange("(p b) -> p b", p=D + 1),
                in_=stg)
            nc.gpsimd.collective_compute(
                "AllGather", ALU.bypass, replica_groups=[list(range(NCORES))],
                ins=[agin[s + 1].ap().opt()], outs=[agout[s + 1].ap().opt()])
            nx2locn2 = sb2.tile([1, BL], BF16, tag="x2locn2", name=f"x2n2_{s + 1}")
            nc.vector.tensor_copy(nx2locn2, stg[D:D + 1, :])
            x_loc, xT_loc = new_x, nxT_loc
            xT_locN2 = stg[0:D, :]
            x2locn2 = nx2locn2

        # ---- stale bandwidth for step s+2: d2f subsample + sqrt-free chain ----
        if 1 <= s <= NB - 2:
            d2f_ps = ps_d2f.tile([128, B], F32, tag="d2f", name=f"d2f{s}")
            nc.tensor.matmul(d2f_ps, lhsT=xall[:, 0:2, :],
                             rhs=xall.rearrange("d c b -> d (c b)"),
                             start=True, stop=False)
            nc.tensor.matmul(d2f_ps, lhsT=ones_row_bf[0:1, 0:128], rhs=x2rowN2,
                             start=False, stop=True)
            x2c_ps = ps_small.tile([128, 1], F32, tag="sm", name=f"x2c{s}")
            nc.tensor.matmul(x2c_ps, lhsT=x2rowN2[0:1, 0:128],
                             rhs=ones_col_bf[0:1, 0:1], start=True, stop=True)
            x2colP = sb3.tile([128, 1], F32, tag="x2colP", name=f"x2colP{s}")
            nc.vector.tensor_scalar(x2colP, x2c_ps, -0.5, None, ALU.mult)
            dsums = sb3.tile([128, 2], F32, tag="dsums", name=f"dsums{s}")
            zscr = scratch.tile([128, B], BF16, tag="zscr", name=f"zscr{s}")
            zscr2 = scratch.tile([128, B], BF16, tag="zscr2", name=f"zscr2{s}")
            # act (exp table, Identity+Square are in every table): z, z^2 sums
            nc.scalar.activation(zscr, d2f_ps, AF.Identity, bias=x2colP,
                                 scale=-0.5, accum_out=dsums[:, 0:1])
            nc.scalar.activation(zscr2, d2f_ps, AF.Square, bias=x2colP,
                                 scale=-0.5, accum_out=dsums[:, 1:2])
            sum12_ps = ps_small.tile([1, 2], F32, tag="sm", name=f"sum12{s}")
            nc.tensor.matmul(sum12_ps, lhsT=ones_col, rhs=dsums, start=True,
                             stop=True)
            # scalar chain: m=(S0/RS), m2=(S1/RS); q=(m2/m^2-1)/8
            # P = m*(1-q)^2 ~ E[sqrt(z)]^2 ; hL = P - A + A^2/(4P); bc=row4/hL
            mrow = sb3.tile([1, 2], F32, tag="mrow", name=f"mrow{s}")
            nc.vector.tensor_scalar(mrow, sum12_ps, 1.0 / float(RSUB), None,
                                    ALU.mult)
            msq = sb3.tile([1, 1], F32, tag="msq", name=f"msq{s}")
            nc.gpsimd.tensor_tensor(msq, mrow[0:1, 0:1], mrow[0:1, 0:1], ALU.mult)
            rmsq = sb3.tile([1, 1], F32, tag="rmsq", name=f"rmsq{s}")
            nc.vector.reciprocal(rmsq, msq)
            t2m = sb3.tile([1, 1], F32, tag="t2m", name=f"t2m{s}")
            nc.gpsimd.tensor_tensor(t2m, mrow[0:1, 1:2], rmsq, ALU.mult)
            uq = sb3.tile([1, 1], F32, tag="uq", name=f"uq{s}")
            nc.gpsimd.tensor_scalar(uq, t2m, -0.125, 1.125, ALU.mult, ALU.add)
            uq2 = sb3.tile([1, 1], F32, tag="uq2", name=f"uq2{s}")
            nc.gpsimd.tensor_tensor(uq2, uq, uq, ALU.mult)
            Pm = sb3.tile([1, 1], F32, tag="Pm", name=f"Pm{s}")
            nc.gpsimd.tensor_tensor(Pm, uq2, mrow[0:1, 0:1], ALU.mult)
            rP = sb3.tile([1, 1], F32, tag="rP", name=f"rP{s}")
            nc.vector.reciprocal(rP, Pm)
            z1 = sb3.tile([1, 1], F32, tag="z1", name=f"z1{s}")
            nc.gpsimd.tensor_scalar(z1, rP, EPS_A * EPS_A / 4.0, -EPS_A,
                                    ALU.mult, ALU.add)
            hL = sb3.tile([1, 1], F32, tag="hL", name=f"hL{s}")
            nc.gpsimd.tensor_tensor(hL, z1, Pm, ALU.add)
            rhL = sb3.tile([1, 1], F32, tag="rhL", name=f"rhL{s}")
            nc.vector.reciprocal(rhL, hL)
            rep4 = sb3.tile([1, 4], F32, tag="rep4", name=f"rep4{s}")
            nc.gpsimd.tensor_scalar(rep4, row4_sb, rhL, None, ALU.mult)
            nbc_ps = ps_small.tile([128, 4], F32, tag="sm", name=f"nbc{s}")
            nc.tensor.matmul(nbc_ps, lhsT=ones_row, rhs=rep4, start=True,
                             stop=True)
            nbc = sb2.tile([128, 4], F32, tag="bc", name=f"bc{s + 2}")
            nc.vector.tensor_copy(nbc, nbc_ps)
            bc_next = nbc
        # (s == 0 keeps bc_next = bc0 for step 1; s == NB-1 ends the loop)

    # ---- HAM warm-keeper: always-ready junk matmuls, issued last so the
    # scheduler only runs them when no real tensor op is ready. Keeps the
    # PE activity window busy -> 2.4 GHz instead of the cold 1.2 GHz.
    junk_ps = ps_junk.tile([128, 512], F32, tag="junk")
    for j in range(NJUNK):
        h = 256 * (j % 2)
        nc.tensor.matmul(junk_ps[:, h:h + 256],
                         lhsT=ones_row_bf[0:1, 0:128],
                         rhs=te_bf[0:1, 0:256], start=True, stop=True)


# ======================================================================
# Host-side wrapper: prep + shard inputs, run SPMD on 8 cores, gather.
# ======================================================================
_CACHE = {}


def _get_nc():
    if "nc" not in _CACHE:
        _CACHE["nc"] = build_nc()
    return _CACHE["nc"]


def _np_gelu(x):
    return 0.5 * x * (1.0 + np.tanh(np.sqrt(2.0 / np.pi)
                                    * (x + 0.044715 * x ** 3)))


def _prep(inputs):
    """Host-side input-only transforms shared by all cores."""
    import ml_dtypes
    bf16 = ml_dtypes.bfloat16
    f32 = np.float32
    g = {}
    dt = float(np.asarray(inputs["eps"], np.float64)[0])
    x0 = np.asarray(inputs["particles"], np.float64)          # [B, D]

    # betas
    sig = 1.0 / (1.0 + np.exp(-np.asarray(inputs["grid_t"], np.float64)))
    betas = np.concatenate([[0.0], np.cumsum(sig)]) / sig.sum()

    # time-embedding table: te_s + in_b  [NB, C]
    coeff = np.linspace(0.1, 100.0, C, dtype=np.float64)[None, :]
    phase = np.asarray(inputs["phase"], np.float64)
    tW1 = np.asarray(inputs["t_W1"], np.float64)
    tW2 = np.asarray(inputs["t_W2"], np.float64)
    TE = np.zeros((NB, C))
    for s in range(NB):
        emb = coeff * s + phase
        temb = np.concatenate([np.sin(emb), np.cos(emb)], -1)
        te = _np_gelu(temb @ tW1 + np.asarray(inputs["t_b1"], np.float64)) \
            @ tW2 + np.asarray(inputs["t_b2"], np.float64)
        TE[s] = te + np.asarray(inputs["in_b"], np.float64)
    g["te_bf"] = TE.astype(f32).astype(bf16).reshape(1, NB * C)

    g["inWs_bf"] = (-0.5 * np.asarray(inputs["in_W"], f32)).astype(bf16)
    hW = np.asarray(inputs["h_W"], f32)                        # [NH, C, C]
    g["hW_bf"] = np.ascontiguousarray(
        hW.reshape(NH, KB, 128, C).transpose(2, 0, 1, 3).reshape(128, -1)
    ).astype(bf16)
    g["hb_bf"] = np.asarray(inputs["h_b"], f32).astype(bf16).reshape(1, NH * C)
    outW = np.asarray(inputs["out_W"], f32)                    # [C, D]
    g["outWs_bf"] = np.ascontiguousarray(
        (dt * outW).reshape(KB, 128, D).transpose(1, 0, 2).reshape(128, -1)
    ).astype(bf16)
    g["outbs_bf"] = (dt * np.asarray(inputs["out_b"], f32)[None, :]).astype(bf16)

    means = np.asarray(inputs["target_means"], f32)
    g["means"] = means
    g["meansT"] = np.ascontiguousarray(means.T)
    g["negmu2"] = (-0.5 * (means.astype(np.float64) ** 2).sum(-1)[None, :]
                   ).astype(f32)
    g["dtb8"] = np.tile((-dt * betas[:NB]).astype(f32)[None, :], (M, 1))
    row4 = np.array([[0.5 * LOGN, EPS_A * LOGN, -0.05 * dt * LOGN,
                      0.1 * dt * LOGN]], np.float64)
    g["row4"] = row4.astype(f32)

    # host bandwidth for steps 0 and 1: hL = h*logn from x0 subsample
    f = x0[:128, None, :] - x0[None, :, :]
    z = (f * f).sum(-1) + EPS_A
    m, m2 = z.mean(), (z * z).mean()
    q = (m2 - m * m) / (8.0 * m * m)
    P = m * (1.0 - q) ** 2
    hL0 = P - EPS_A + EPS_A * EPS_A / (4.0 * P)
    g["bcast0_row"] = (row4 / hL0).astype(f32)
    g["omd_col"] = np.full((128, 1), 1.0 - dt, f32)

    # full-particle tiles for step 0
    x0f = x0.astype(f32)
    g["xall0"] = np.ascontiguousarray(-2.0 * x0f.T).astype(bf16)
    x2 = (x0f * x0f).sum(-1) + 1.0
    g["x2rowN2_0"] = (-2.0 * x2[None, :]).astype(bf16)

    # noise, pre-scaled, [B, NB, D]
    noi = np.asarray(inputs["noises"], f32) * np.float32(np.sqrt(2.0 * dt))
    g["noises_all"] = np.ascontiguousarray(noi.transpose(1, 0, 2))
    g["x0f"] = x0f
    return g


def _shard(g, c):
    import ml_dtypes
    bf16 = ml_dtypes.bfloat16
    sl = slice(c * BL, (c + 1) * BL)
    m = {k: g[k] for k in ["inWs_bf", "te_bf", "hW_bf", "hb_bf", "outWs_bf",
                           "outbs_bf", "means", "meansT", "negmu2", "dtb8",
                           "row4", "bcast0_row", "omd_col", "xall0",
                           "x2rowN2_0"]}
    x0l = np.ascontiguousarray(g["x0f"][sl])
    m["x0_loc"] = x0l
    m["xT0_loc"] = np.ascontiguousarray(x0l.T)
    m["xT0_locN2"] = np.ascontiguousarray(-2.0 * x0l.T).astype(bf16)
    x2 = (x0l * x0l).sum(-1) + 1.0
    m["x2locn2_0"] = (-2.0 * x2[None, :]).astype(bf16)
    m["noises_s"] = np.ascontiguousarray(g["noises_all"][sl])
    return m


def run(inputs, trace=False, trace_cores=None):
    from concourse.bass_utils import run_bass_kernel_spmd
    nc = _get_nc()
    g = _prep(inputs)
    in_maps = [_shard(g, c) for c in range(NCORES)]
    res = run_bass_kernel_spmd(nc, in_maps, core_ids=list(range(NCORES)),
                               trace=trace, trace_cores=trace_cores)
    out = np.zeros((NB + 1, B, D), np.float32)
    out[0] = np.asarray(inputs["particles"], np.float32)
    for c in range(NCORES):
        out[1:, c * BL:(c + 1) * BL, :] = \
            np.asarray(res.results[c]["traj"]).reshape(NB, BL, D)
    return out, res


def kernel(**inputs):
    return run(inputs)[0]


# revision 19
# speedup vs baseline: 1.1492x; 1.1492x over previous
"""Bass/Tile kernel for nn_CMCD (annealed Langevin sampler with SVGD repulsion).

SPMD over 8 cores, data-parallel over the particle batch (64 rows/core).

Structure (v2):
- Host precomputes all input-only transforms: time-embedding table, betas,
  weight layouts/casts, noise prescale, and the step-0 particle tiles
  (so step 0 needs no collective).
- A tiny warm-up AllGather fires at t~0 to absorb collective mesh-init /
  core-start skew while weights stream in.
- Per step s>=1: AllGather of x_s posted at the end of step s-1; the
  score net + mixture-gradient run in its shadow; repulsion from the
  gathered particles; fused update.
- Activation-table discipline: steady-state act functions are only
  {Exp, Gelu, Identity, Square} ordered as [exp-block][gelu-block] per
  step -> 2 table loads/step.
- Bandwidth (SVGD median heuristic) replaced by a calibrated sqrt-free
  estimator computed from mean/var of d2 over a 128x512 subsample, one
  step stale (h_s = h(x_{s-1})); step 0/1 bandwidth comes from the host.
  Validated end-to-end at rel err ~2e-5 vs the jax reference.
"""
import numpy as np
from contextlib import ExitStack

import concourse.bass as bass
import concourse.bacc as bacc
import concourse.tile as tile
from concourse import mybir
from concourse.masks import make_identity

D, C, NB, NH, M = 64, 512, 8, 3, 8
B = 512
NCORES = 8
BL = B // NCORES  # 64
KB = C // 128     # 4 channel blocks
LOGN = float(np.log(B))
RSUB = 128 * B    # subsample count for the bandwidth (rows 0..127)
AGW = BL * D + BL  # flat AllGather payload words per core
NJUNK = 400
EPS_A = 2.0        # total d2 shift (bf16-safety); corrected exactly
F32 = mybir.dt.float32
BF16 = mybir.dt.bfloat16
AF = mybir.ActivationFunctionType
ALU = mybir.AluOpType
GELU = AF.Gelu_apprx_tanh


def build_nc(compile=True):
    nc = bacc.Bacc("TRN2", target_bir_lowering=False, debug=False,
                   num_devices=NCORES)

    t = {}
    def din(name, shape, dtype):
        t[name] = nc.dram_tensor(name, shape, dtype, kind="ExternalInput")

    # ---- per-core state inputs ----
    din("x0_loc", [BL, D], F32)
    din("xT0_loc", [D, BL], F32)
    din("xT0_locN2", [D, BL], BF16)
    din("x2locn2_0", [1, BL], BF16)
    din("xall0", [D, B], BF16)        # -2 * x0^T, all particles
    din("x2rowN2_0", [1, B], BF16)    # -2*(|x0_j|^2 + 1), all particles
    din("noises_s", [BL, NB, D], F32)  # pre-scaled by sqrt(2 dt)
    # ---- weights / tables (host-prepped) ----
    din("inWs_bf", [D, C], BF16)       # -0.5 * in_W
    din("te_bf", [1, NB * C], BF16)    # te_s + in_b, flat row
    din("hW_bf", [128, NH * KB * C], BF16)
    din("hb_bf", [1, NH * C], BF16)
    din("outWs_bf", [128, KB * D], BF16)  # dt * out_W
    din("outbs_bf", [1, D], BF16)         # dt * out_b
    din("means", [M, D], F32)
    din("meansT", [D, M], F32)
    din("negmu2", [1, M], F32)
    din("dtb8", [M, NB], F32)          # col s = -dt*beta_s
    din("row4", [1, 4], F32)           # [.5*logn, A*logn, -.05*dt*logn, .1*dt*logn]
    din("bcast0_row", [1, 4], F32)     # row4 / hL(x0)
    din("omd_col", [128, 1], F32)      # 1 - dt

    traj_d = nc.dram_tensor("traj", [NB, BL, D], F32, kind="ExternalOutput")
    t["traj_d"] = traj_d

    # collective bounce buffers: steps 1..NB-1, plus a warm-up dummy
    t["agin"] = [None] + [nc.dram_tensor(f"agin{s}", [AGW], BF16)
                          for s in range(1, NB)]
    t["agout"] = [None] + [nc.dram_tensor(f"agout{s}", [NCORES, AGW], BF16,
                                          addr_space="Shared")
                           for s in range(1, NB)]
    t["dd_in"] = nc.dram_tensor("dd_in", [64], BF16)
    t["dd_out"] = nc.dram_tensor("dd_out", [NCORES, 64], BF16,
                                 addr_space="Shared")

    with tile.TileContext(nc) as tc, ExitStack() as ctx:
        _body(ctx, tc, nc, t)
    if compile:
        nc.compile()
    return nc


def _body(ctx, tc, nc, t):
    traj_d, agin, agout = t["traj_d"], t["agin"], t["agout"]

    const = ctx.enter_context(tc.tile_pool(name="const", bufs=1))
    wpool = ctx.enter_context(tc.tile_pool(name="wpool", bufs=1))
    sb2 = ctx.enter_context(tc.tile_pool(name="sb2", bufs=2))
    sb3 = ctx.enter_context(tc.tile_pool(name="sb3", bufs=3))
    scratch = ctx.enter_context(tc.tile_pool(name="scratch", bufs=2))
    ps_small = ctx.enter_context(tc.tile_pool(name="ps_small", bufs=2, space="PSUM"))
    ps_d2f = ctx.enter_context(tc.tile_pool(name="ps_d2f", bufs=1, space="PSUM"))
    ps_d2l = ctx.enter_context(tc.tile_pool(name="ps_d2l", bufs=1, space="PSUM"))
    ps_u = ctx.enter_context(tc.tile_pool(name="ps_u", bufs=1, space="PSUM"))
    ps_cmp = ctx.enter_context(tc.tile_pool(name="ps_cmp", bufs=1, space="PSUM"))
    ps_net = ctx.enter_context(tc.tile_pool(name="ps_net", bufs=2, space="PSUM"))

    # ---------------- warm-up collective: very first instruction ----------------
    nc.gpsimd.collective_compute(
        "AllGather", ALU.bypass, replica_groups=[list(range(NCORES))],
        ins=[t["dd_in"].ap().opt()], outs=[t["dd_out"].ap().opt()])

    # ---------------- input DMAs (3 queues, ordered by first use) ----------------
    # queue A (sync): step-0 particle tiles + noises
    x0_loc = wpool.tile([BL, D], F32)
    nc.sync.dma_start(out=x0_loc, in_=t["x0_loc"][:, :])
    xT0_loc = wpool.tile([D, BL], F32)
    nc.sync.dma_start(out=xT0_loc, in_=t["xT0_loc"][:, :])
    xT0_locN2 = wpool.tile([D, BL], BF16)
    nc.sync.dma_start(out=xT0_locN2, in_=t["xT0_locN2"][:, :])
    x2locn2_0 = wpool.tile([1, BL], BF16)
    nc.sync.dma_start(out=x2locn2_0, in_=t["x2locn2_0"][:, :])
    xall0 = wpool.tile([D, NCORES, BL], BF16)
    nc.sync.dma_start(out=xall0, in_=t["xall0"].ap().rearrange(
        "d (c b) -> d c b", c=NCORES))
    x2rowN2_0 = wpool.tile([1, B], BF16)
    nc.sync.dma_start(out=x2rowN2_0, in_=t["x2rowN2_0"][:, :])
    noise_sb = wpool.tile([BL, NB, D], F32)
    nc.sync.dma_start(out=noise_sb, in_=t["noises_s"][:, :, :])
    # queue A continues: second half of hW
    hW_sb = wpool.tile([128, NH, KB, C], BF16)
    # queue B (scalar): small weights in use order
    inWs_bf = wpool.tile([D, C], BF16)
    nc.scalar.dma_start(out=inWs_bf, in_=t["inWs_bf"][:, :])
    te_bf = wpool.tile([1, NB * C], BF16)
    nc.scalar.dma_start(out=te_bf[0:1, 0:NB * C // 2],
                        in_=t["te_bf"][0:1, 0:NB * C // 2])
    nc.sync.dma_start(out=te_bf[0:1, NB * C // 2:],
                      in_=t["te_bf"][0:1, NB * C // 2:])
    meansT_sb = wpool.tile([D, M], F32)
    nc.scalar.dma_start(out=meansT_sb, in_=t["meansT"][:, :])
    negmu2_row = wpool.tile([1, M], F32)
    nc.scalar.dma_start(out=negmu2_row, in_=t["negmu2"][:, :])
    means_sb = wpool.tile([M, D], F32)
    nc.scalar.dma_start(out=means_sb, in_=t["means"][:, :])
    dtb8_sb = wpool.tile([M, NB], F32)
    nc.scalar.dma_start(out=dtb8_sb, in_=t["dtb8"][:, :])
    row4_sb = wpool.tile([1, 4], F32)
    nc.scalar.dma_start(out=row4_sb, in_=t["row4"][:, :])
    bcast0_row = wpool.tile([1, 4], F32)
    nc.scalar.dma_start(out=bcast0_row, in_=t["bcast0_row"][:, :])
    omd_col = wpool.tile([128, 1], F32)
    nc.scalar.dma_start(out=omd_col, in_=t["omd_col"][:, :])
    hb_sb = wpool.tile([1, NH * C], BF16)
    nc.scalar.dma_start(out=hb_sb, in_=t["hb_bf"][:, :])
    outWs_sb = wpool.tile([128, KB, D], BF16)
    nc.scalar.dma_start(out=outWs_sb, in_=t["outWs_bf"].ap().rearrange(
        "p (k d) -> p k d", k=KB))
    outbs_row = wpool.tile([1, D], BF16)
    nc.scalar.dma_start(out=outbs_row, in_=t["outbs_bf"][:, :])
    # hW split across queues A and B (~750KB each)
    hWr = t["hW_bf"].ap().rearrange("p (l k c) -> p l k c", l=NH, k=KB)
    nc.scalar.dma_start(out=hW_sb[:, 0:2, :, :], in_=hWr[:, 0:2, :, :])
    nc.sync.dma_start(out=hW_sb[:, 2:NH, :, :], in_=hWr[:, 2:NH, :, :])

    # ---------------- constants ----------------
    ident = const.tile([128, 128], F32)
    make_identity(nc, ident)
    ident_bf = const.tile([128, 128], BF16)
    nc.vector.tensor_copy(ident_bf, ident)
    ones_col = const.tile([128, 1], F32)
    nc.vector.memset(ones_col, 1.0)
    ones_row = const.tile([1, 128], F32)
    nc.vector.memset(ones_row, 1.0)
    ones_row_bf = const.tile([1, 128], BF16)
    nc.vector.memset(ones_row_bf, 1.0)
    ones_col_bf = const.tile([128, 1], BF16)
    nc.vector.memset(ones_col_bf, 1.0)

    # broadcast bcast0_row -> [128, 4] (used by steps 0 and 1)
    bc0_ps = ps_small.tile([128, 4], F32, tag="sm", name="bc0_ps")
    nc.tensor.matmul(bc0_ps, lhsT=ones_row, rhs=bcast0_row, start=True, stop=True)
    bc0 = const.tile([128, 4], F32)
    nc.vector.tensor_copy(bc0, bc0_ps)

    # ---------------- per-step state handles ----------------
    x_loc = x0_loc
    xT_loc = xT0_loc
    xT_locN2 = xT0_locN2
    x2locn2 = x2locn2_0
    bc_next = bc0  # bandwidth broadcast for the *next* issued step

    for s in range(NB):
        bc = bc_next
        # ---- gathered particle tiles ----
        if s == 0:
            xall = xall0
            x2rowN2 = x2rowN2_0
        else:
            xall = sb2.tile([D, NCORES, BL], BF16, tag="xall", name=f"xall{s}")
            for half, eng in ((0, nc.sync), (1, nc.scalar)):
                eng.dma_start(
                    out=xall[:, half * 4:(half + 1) * 4, :],
                    in_=bass.AP(tensor=agout[s].ap().tensor,
                                offset=half * 4 * AGW,
                                ap=[[BL, D], [AGW, 4], [1, BL]]))
            x2rowN2 = sb3.tile([1, B], BF16, tag="x2rowN2", name=f"x2r{s}")
            nc.sync.dma_start(
                out=x2rowN2.rearrange("o (c b) -> o c b", c=NCORES),
                in_=bass.AP(tensor=agout[s].ap().tensor, offset=BL * D,
                            ap=[[0, 1], [AGW, NCORES], [1, BL]]))

        # ================= TENSOR (+ matching act/vector) =================
        # ---- mixture-component logits (feeds softmax exp, first act op) ----
        comp_ps = ps_cmp.tile([BL, M], F32, tag="cmp", name=f"comp{s}")
        nc.tensor.matmul(comp_ps, lhsT=xT_loc, rhs=meansT_sb, start=True, stop=False)
        nc.tensor.matmul(comp_ps, lhsT=ones_row[0:1, 0:BL], rhs=negmu2_row,
                         start=False, stop=True)
        negmax = sb3.tile([BL, 1], F32, tag="negmax", name=f"negmax{s}")
        nc.vector.tensor_reduce(negmax, comp_ps, axis=mybir.AxisListType.X,
                                op=ALU.max, negate=True)
        # softmax exps via Tanh (resident in the gelu table):
        # e^-w = (1-tanh(w/2))/(1+tanh(w/2)), w = -(comp+negmax) >= 0
        halfmax = sb3.tile([BL, 1], F32, tag="halfmax", name=f"halfmax{s}")
        nc.vector.tensor_scalar(halfmax, negmax, -0.5, None, ALU.mult)
        tsm = sb3.tile([BL, M], F32, tag="tsm", name=f"tsm{s}")
        nc.scalar.activation(tsm, comp_ps, AF.Tanh, bias=halfmax, scale=-0.5)
        numt = sb3.tile([BL, M], F32, tag="numt", name=f"numt{s}")
        nc.vector.tensor_scalar(numt, tsm, -1.0, 1.0, ALU.mult, ALU.add)
        dent = sb3.tile([BL, M], F32, tag="dent", name=f"dent{s}")
        nc.vector.tensor_scalar(dent, tsm, 1.0, 1.0, ALU.mult, ALU.add)
        rden = sb3.tile([BL, M], F32, tag="rden", name=f"rden{s}")
        nc.vector.reciprocal(rden, dent)
        w_un = sb3.tile([BL, M], F32, tag="w_un", name=f"w_un{s}")
        nc.vector.tensor_tensor(w_un, numt, rden, ALU.mult)
        sumexp = sb3.tile([BL, 1], F32, tag="sumexp", name=f"sumexp{s}")
        nc.vector.tensor_reduce(sumexp, w_un, axis=mybir.AxisListType.X,
                                op=ALU.add)
        rcp = sb3.tile([BL, 1], F32, tag="rcp", name=f"rcp{s}")
        nc.vector.reciprocal(rcp, sumexp)
        w_n = sb3.tile([BL, M], F32, tag="w_n", name=f"w_n{s}")
        nc.vector.tensor_scalar(w_n, w_un, rcp, None, ALU.mult)

        # ---- score net layer 1: h^T = (x @ in_W)^T + te (gelu) ----
        h_ps = ps_net.tile([128, KB, BL], F32, tag="h_ps", bufs=1, name=f"h_ps{s}")
        for ko in range(KB):
            nc.tensor.matmul(h_ps[:, ko, :],
                             lhsT=inWs_bf[:, 128 * ko:128 * (ko + 1)],
                             rhs=xT_locN2, start=True, stop=False)
            nc.tensor.matmul(h_ps[:, ko, :],
                             lhsT=te_bf[0:1, s * C + 128 * ko:
                                        s * C + 128 * (ko + 1)],
                             rhs=ones_row_bf[0:1, 0:BL], start=False, stop=True)
        h_sb = sb2.tile([128, KB, BL], BF16, tag="h0", name=f"h0_{s}")
        # act #2: Gelu (table load; net has slack vs the collective path)
        nc.scalar.activation(h_sb.rearrange("p k b -> p (k b)"),
                             h_ps.rearrange("p k b -> p (k b)"), GELU)

        # ---- softmax tail on tensor (w^T, scaled by -dt*beta_s) ----
        wT_ps = ps_small.tile([M, BL], F32, tag="sm", name=f"wT{s}")
        nc.tensor.transpose(wT_ps, w_n, ident[0:BL, 0:BL])
        wTs_sb = sb3.tile([M, BL], F32, tag="wTs", name=f"wTs{s}")
        nc.vector.tensor_scalar(wTs_sb, wT_ps, dtb8_sb[0:M, s:s + 1], None, ALU.mult)

        # ---- hidden layers ----
        for l in range(NH):
            hu_ps = ps_net.tile([BL, C], F32, tag="hu", bufs=1, name=f"hu{s}_{l}")
            for ki in range(KB):
                nc.tensor.matmul(hu_ps, lhsT=h_sb[:, ki, :], rhs=hW_sb[:, l, ki, :],
                                 start=(ki == 0), stop=False)
            nc.tensor.matmul(hu_ps, lhsT=ones_row_bf[0:1, 0:BL],
                             rhs=hb_sb[0:1, l * C:(l + 1) * C],
                             start=False, stop=True)
            hu_sb = sb2.tile([BL, C], BF16, tag="hu_sb", name=f"hu_sb{s}_{l}")
            nc.vector.tensor_copy(hu_sb, hu_ps)
            tps = ps_net.tile([128, KB, BL], BF16, tag="h_ps", bufs=1,
                              name=f"tps{s}_{l}")
            for k in range(KB):
                nc.tensor.transpose(tps[:, k, :], hu_sb[:, 128 * k:128 * (k + 1)],
                                    ident_bf[0:BL, 0:BL])
            hn_sb = sb2.tile([128, KB, BL], BF16, tag=f"h{l + 1}",
                             name=f"hn_sb{s}_{l}")
            nc.scalar.activation(hn_sb.rearrange("p k b -> p (k b)"),
                                 tps.rearrange("p k b -> p (k b)"), GELU)
            h_sb = hn_sb

        # ---- pairwise d2, local columns: psum = -2*(d2+A) ----
        d2l_ps = ps_d2l.tile([128, KB, BL], F32, tag="d2l", name=f"d2l{s}")
        for k in range(KB):
            nc.tensor.matmul(d2l_ps[:, k, :], lhsT=xall[:, 2 * k:2 * k + 2, :],
                             rhs=xT_locN2, start=True, stop=False)
            nc.tensor.matmul(d2l_ps[:, k, :], lhsT=ones_row_bf[0:1, 0:128],
                             rhs=x2locn2, start=False, stop=False)
            nc.tensor.matmul(d2l_ps[:, k, :],
                             lhsT=x2rowN2[0:1, 128 * k:128 * (k + 1)],
                             rhs=ones_row_bf[0:1, 0:BL], start=False, stop=True)

        # ---- repulsion kernel: kt = exp(-d2/h) ----
        kt_sb = sb2.tile([128, KB, BL], BF16, tag="kt", name=f"kt{s}")
        # act: Exp (table load #2; hides under the collective/d2l window)
        nc.scalar.activation(kt_sb.rearrange("p k b -> p (k b)"),
                             d2l_ps.rearrange("p k b -> p (k b)"), AF.Exp,
                             bias=bc[:, 1:2], scale=bc[:, 0:1])

        # ---- x rows scaled by c_h: xfe = c_h * x  (from -2x^T blocks) ----
        xft_ps = ps_net.tile([128, KB, BL], BF16, tag="h_ps", bufs=1,
                             name=f"xft{s}")
        for k in range(KB):
            nc.tensor.transpose(xft_ps[:, k, :], xall[:, 2 * k:2 * k + 2, :],
                                ident_bf[0:D, 0:D])
        xfe = sb2.tile([128, KB, BL], BF16, tag="xfe", name=f"xfe{s}")
        nc.vector.tensor_scalar(xfe.rearrange("p k b -> p (k b)"),
                                xft_ps.rearrange("p k b -> p (k b)"),
                                bc[:, 2:3], None, ALU.mult)

        # ---- U = dt*score + dt*out_b - dt*beta*(w@means) ----
        ur_ps = ps_u.tile([BL, 2, D], F32, tag="u", name=f"ur{s}")
        u_ps = ur_ps[:, 0, :]
        for ki in range(KB):
            nc.tensor.matmul(u_ps, lhsT=h_sb[:, ki, :], rhs=outWs_sb[:, ki, :],
                             start=(ki == 0), stop=False)
        nc.tensor.matmul(u_ps, lhsT=ones_row_bf[0:1, 0:BL], rhs=outbs_row,
                         start=False, stop=False)
        nc.tensor.matmul(u_ps, lhsT=wTs_sb, rhs=means_sb, start=False, stop=True)

        # ---- kxr = c_h * K@x ; rch = c_h * r ----
        kxr_ps = ur_ps[:, 1, :]
        for k in range(KB):
            nc.tensor.matmul(kxr_ps, lhsT=kt_sb[:, k, :], rhs=xfe[:, k, :],
                             start=(k == 0), stop=(k == KB - 1))
        chcol_bf = sb3.tile([128, 1], BF16, tag="chcol", name=f"chcol{s}")
        nc.vector.tensor_copy(chcol_bf, bc[:, 3:4])
        rch_ps = ps_small.tile([BL, 1], F32, tag="sm", name=f"rch{s}")
        for k in range(KB):
            nc.tensor.matmul(rch_ps, lhsT=kt_sb[:, k, :], rhs=chcol_bf,
                             start=(k == 0), stop=(k == KB - 1))

        # ---- update: new = x*(1-dt+c_h*r) + noise - U - c_h*K@x ----
        alpha = sb3.tile([BL, 1], F32, tag="alpha", name=f"alpha{s}")
        nc.vector.tensor_tensor(alpha, rch_ps, omd_col[0:BL, 0:1], ALU.add)
        t1 = sb3.tile([BL, D], F32, tag="t1", name=f"t1_{s}")
        nc.vector.tensor_scalar(t1, x_loc, alpha, None, ALU.mult)
        t2 = sb3.tile([BL, D], F32, tag="t2", name=f"t2_{s}")
        nc.vector.tensor_tensor(t2, t1, noise_sb[:, s, :], ALU.add)
        t3 = sb3.tile([BL, D], F32, tag="t3", name=f"t3_{s}")
        nc.vector.tensor_tensor(t3, t2, u_ps, ALU.subtract)
        new_x = sb2.tile([BL, D], F32, tag="x_loc", name=f"x{s + 1}")
        nc.vector.tensor_tensor(new_x, t3, kxr_ps, ALU.subtract)
        nc.scalar.dma_start(out=traj_d[s], in_=new_x)

        # ---- stage + post AllGather for step s+1 (one DMA: [-2x^T | x2]) ----
        if s + 1 < NB:
            nxT_ps = ps_small.tile([D, BL], F32, tag="sm", name=f"nxT{s + 1}")
            nc.tensor.transpose(nxT_ps, new_x, ident[0:BL, 0:BL])
            stg = sb2.tile([D + 1, BL], BF16, tag="stg", name=f"stg{s + 1}")
            nc.vector.tensor_scalar(stg[0:D, :], nxT_ps, -2.0, None, ALU.mult)
            nxT_loc = sb2.tile([D, BL], F32, tag="xT_loc", name=f"xT{s + 1}")
            nc.vector.tensor_copy(nxT_loc, nxT_ps)
            sqnT = scratch.tile([D, BL], F32, tag="sqnT", name=f"sqnT{s + 1}")
            nc.vector.tensor_tensor(sqnT, nxT_ps, nxT_loc, ALU.mult)
            x2l_ps = ps_small.tile([1, BL], F32, tag="sm", name=f"x2l{s + 1}")
            nc.tensor.matmul(x2l_ps, lhsT=ones_col[0:D, 0:1], rhs=sqnT,
                             start=True, stop=True)
            nc.vector.tensor_scalar(stg[D:D + 1, :], x2l_ps, -2.0, -2.0,
                                    ALU.mult, ALU.add)
            nc.sync.dma_start(
                out=agin[s + 1].ap()[0:AGW].rearrange("(p b) -> p b", p=D + 1),
                in_=stg)
            nc.gpsimd.collective_compute(
                "AllGather", ALU.bypass, replica_groups=[list(range(NCORES))],
                ins=[agin[s + 1].ap().opt()], outs=[agout[s + 1].ap().opt()])
            nx2locn2 = sb2.tile([1, BL], BF16, tag="x2locn2", name=f"x2n2_{s + 1}")
            nc.vector.tensor_copy(nx2locn2, stg[D:D + 1, :])
            x_loc, xT_loc = new_x, nxT_loc
            xT_locN2 = stg[0:D, :]
            x2locn2 = nx2locn2

        # ---- stale bandwidth for step s+2: d2f subsample + sqrt-free chain ----
        if 1 <= s <= NB - 2:
            d2f_ps = ps_d2f.tile([128, B], F32, tag="d2f", name=f"d2f{s}")
            nc.tensor.matmul(d2f_ps, lhsT=xall[:, 0:2, :],
                             rhs=xall.rearrange("d c b -> d (c b)"),
                             start=True, stop=False)
            nc.tensor.matmul(d2f_ps, lhsT=ones_row_bf[0:1, 0:128], rhs=x2rowN2,
                             start=False, stop=True)
            x2c_ps = ps_small.tile([128, 1], F32, tag="sm", name=f"x2c{s}")
            nc.tensor.matmul(x2c_ps, lhsT=x2rowN2[0:1, 0:128],
                             rhs=ones_col_bf[0:1, 0:1], start=True, stop=True)
            x2colP = sb3.tile([128, 1], F32, tag="x2colP", name=f"x2colP{s}")
            nc.vector.tensor_scalar(x2colP, x2c_ps, -0.5, None, ALU.mult)
            dsums = sb3.tile([128, 2], F32, tag="dsums", name=f"dsums{s}")
            zscr = scratch.tile([128, B], BF16, tag="zscr", name=f"zscr{s}")
            zscr2 = scratch.tile([128, B], BF16, tag="zscr2", name=f"zscr2{s}")
            # act (exp table, Identity+Square are in every table): z, z^2 sums
            nc.scalar.activation(zscr, d2f_ps, AF.Identity, bias=x2colP,
                                 scale=-0.5, accum_out=dsums[:, 0:1])
            nc.scalar.activation(zscr2, d2f_ps, AF.Square, bias=x2colP,
                                 scale=-0.5, accum_out=dsums[:, 1:2])
            sum12_ps = ps_small.tile([1, 2], F32, tag="sm", name=f"sum12{s}")
            nc.tensor.matmul(sum12_ps, lhsT=ones_col, rhs=dsums, start=True,
                             stop=True)
            # scalar chain: m=(S0/RS), m2=(S1/RS); q=(m2/m^2-1)/8
            # P = m*(1-q)^2 ~ E[sqrt(z)]^2 ; hL = P - A + A^2/(4P); bc=row4/hL
            mrow = sb3.tile([1, 2], F32, tag="mrow", name=f"mrow{s}")
            nc.vector.tensor_scalar(mrow, sum12_ps, 1.0 / float(RSUB), None,
                                    ALU.mult)
            msq = sb3.tile([1, 1], F32, tag="msq", name=f"msq{s}")
            nc.gpsimd.tensor_tensor(msq, mrow[0:1, 0:1], mrow[0:1, 0:1], ALU.mult)
            rmsq = sb3.tile([1, 1], F32, tag="rmsq", name=f"rmsq{s}")
            nc.vector.reciprocal(rmsq, msq)
            t2m = sb3.tile([1, 1], F32, tag="t2m", name=f"t2m{s}")
            nc.gpsimd.tensor_tensor(t2m, mrow[0:1, 1:2], rmsq, ALU.mult)
            uq = sb3.tile([1, 1], F32, tag="uq", name=f"uq{s}")
            nc.gpsimd.tensor_scalar(uq, t2m, -0.125, 1.125, ALU.mult, ALU.add)
            uq2 = sb3.tile([1, 1], F32, tag="uq2", name=f"uq2{s}")
            nc.gpsimd.tensor_tensor(uq2, uq, uq, ALU.mult)
            Pm = sb3.tile([1, 1], F32, tag="Pm", name=f"Pm{s}")
            nc.gpsimd.tensor_tensor(Pm, uq2, mrow[0:1, 0:1], ALU.mult)
            rP = sb3.tile([1, 1], F32, tag="rP", name=f"rP{s}")
            nc.vector.reciprocal(rP, Pm)
            z1 = sb3.tile([1, 1], F32, tag="z1", name=f"z1{s}")
            nc.gpsimd.tensor_scalar(z1, rP, EPS_A * EPS_A / 4.0, -EPS_A,
                                    ALU.mult, ALU.add)
            hL = sb3.tile([1, 1], F32, tag="hL", name=f"hL{s}")
            nc.gpsimd.tensor_tensor(hL, z1, Pm, ALU.add)
            rhL = sb3.tile([1, 1], F32, tag="rhL", name=f"rhL{s}")
            nc.vector.reciprocal(rhL, hL)
            rep4 = sb3.tile([1, 4], F32, tag="rep4", name=f"rep4{s}")
            nc.gpsimd.tensor_scalar(rep4, row4_sb, rhL, None, ALU.mult)
            nbc_ps = ps_small.tile([128, 4], F32, tag="sm", name=f"nbc{s}")
            nc.tensor.matmul(nbc_ps, lhsT=ones_row, rhs=rep4, start=True,
                             stop=True)
            nbc = sb2.tile([128, 4], F32, tag="bc", name=f"bc{s + 2}")
            nc.vector.tensor_copy(nbc, nbc_ps)
            bc_next = nbc
        # (s == 0 keeps bc_next = bc0 for step 1; s == NB-1 ends the loop)


# ======================================================================
# Host-side wrapper: prep + shard inputs, run SPMD on 8 cores, gather.
# ======================================================================
_CACHE = {}


def _get_nc():
    if "nc" not in _CACHE:
        _CACHE["nc"] = build_nc()
    return _CACHE["nc"]


def _np_gelu(x):
    return 0.5 * x * (1.0 + np.tanh(np.sqrt(2.0 / np.pi)
                                    * (x + 0.044715 * x ** 3)))


def _prep(inputs):
    """Host-side input-only transforms shared by all cores."""
    import ml_dtypes
    bf16 = ml_dtypes.bfloat16
    f32 = np.float32
    g = {}
    dt = float(np.asarray(inputs["eps"], np.float64)[0])
    x0 = np.asarray(inputs["particles"], np.float64)          # [B, D]

    # betas
    sig = 1.0 / (1.0 + np.exp(-np.asarray(inputs["grid_t"], np.float64)))
    betas = np.concatenate([[0.0], np.cumsum(sig)]) / sig.sum()

    # time-embedding table: te_s + in_b  [NB, C]
    coeff = np.linspace(0.1, 100.0, C, dtype=np.float64)[None, :]
    phase = np.asarray(inputs["phase"], np.float64)
    tW1 = np.asarray(inputs["t_W1"], np.float64)
    tW2 = np.asarray(inputs["t_W2"], np.float64)
    TE = np.zeros((NB, C))
    for s in range(NB):
        emb = coeff * s + phase
        temb = np.concatenate([np.sin(emb), np.cos(emb)], -1)
        te = _np_gelu(temb @ tW1 + np.asarray(inputs["t_b1"], np.float64)) \
            @ tW2 + np.asarray(inputs["t_b2"], np.float64)
        TE[s] = te + np.asarray(inputs["in_b"], np.float64)
    g["te_bf"] = TE.astype(f32).astype(bf16).reshape(1, NB * C)

    g["inWs_bf"] = (-0.5 * np.asarray(inputs["in_W"], f32)).astype(bf16)
    hW = np.asarray(inputs["h_W"], f32)                        # [NH, C, C]
    g["hW_bf"] = np.ascontiguousarray(
        hW.reshape(NH, KB, 128, C).transpose(2, 0, 1, 3).reshape(128, -1)
    ).astype(bf16)
    g["hb_bf"] = np.asarray(inputs["h_b"], f32).astype(bf16).reshape(1, NH * C)
    outW = np.asarray(inputs["out_W"], f32)                    # [C, D]
    g["outWs_bf"] = np.ascontiguousarray(
        (dt * outW).reshape(KB, 128, D).transpose(1, 0, 2).reshape(128, -1)
    ).astype(bf16)
    g["outbs_bf"] = (dt * np.asarray(inputs["out_b"], f32)[None, :]).astype(bf16)

    means = np.asarray(inputs["target_means"], f32)
    g["means"] = means
    g["meansT"] = np.ascontiguousarray(means.T)
    g["negmu2"] = (-0.5 * (means.astype(np.float64) ** 2).sum(-1)[None, :]
                   ).astype(f32)
    g["dtb8"] = np.tile((-dt * betas[:NB]).astype(f32)[None, :], (M, 1))
    row4 = np.array([[0.5 * LOGN, EPS_A * LOGN, -0.05 * dt * LOGN,
                      0.1 * dt * LOGN]], np.float64)
    g["row4"] = row4.astype(f32)

    # host bandwidth for steps 0 and 1: hL = h*logn from x0 subsample
    f = x0[:128, None, :] - x0[None, :, :]
    z = (f * f).sum(-1) + EPS_A
    m, m2 = z.mean(), (z * z).mean()
    q = (m2 - m * m) / (8.0 * m * m)
    P = m * (1.0 - q) ** 2
    hL0 = P - EPS_A + EPS_A * EPS_A / (4.0 * P)
    g["bcast0_row"] = (row4 / hL0).astype(f32)
    g["omd_col"] = np.full((128, 1), 1.0 - dt, f32)

    # full-particle tiles for step 0
    x0f = x0.astype(f32)
    g["xall0"] = np.ascontiguousarray(-2.0 * x0f.T).astype(bf16)
    x2 = (x0f * x0f).sum(-1) + 1.0
    g["x2rowN2_0"] = (-2.0 * x2[None, :]).astype(bf16)

    # noise, pre-scaled, [B, NB, D]
    noi = np.asarray(inputs["noises"], f32) * np.float32(np.sqrt(2.0 * dt))
    g["noises_all"] = np.ascontiguousarray(noi.transpose(1, 0, 2))
    g["x0f"] = x0f
    return g


def _shard(g, c):
    import ml_dtypes
    bf16 = ml_dtypes.bfloat16
    sl = slice(c * BL, (c + 1) * BL)
    m = {k: g[k] for k in ["inWs_bf", "te_bf", "hW_bf", "hb_bf", "outWs_bf",
                           "outbs_bf", "means", "meansT", "negmu2", "dtb8",
                           "row4", "bcast0_row", "omd_col", "xall0",
                           "x2rowN2_0"]}
    x0l = np.ascontiguousarray(g["x0f"][sl])
    m["x0_loc"] = x0l
    m["xT0_loc"] = np.ascontiguousarray(x0l.T)
    m["xT0_locN2"] = np.ascontiguousarray(-2.0 * x0l.T).astype(bf16)
    x2 = (x0l * x0l).sum(-1) + 1.0
    m["x2locn2_0"] = (-2.0 * x2[None, :]).astype(bf16)
    m["noises_s"] = np.ascontiguousarray(g["noises_all"][sl])
    return m


def run(inputs, trace=False, trace_cores=None):
    from concourse.bass_utils import run_bass_kernel_spmd
    nc = _get_nc()
    g = _prep(inputs)
    in_maps = [_shard(g, c) for c in range(NCORES)]
    res = run_bass_kernel_spmd(nc, in_maps, core_ids=list(range(NCORES)),
                               trace=trace, trace_cores=trace_cores)
    out = np.zeros((NB + 1, B, D), np.float32)
    out[0] = np.asarray(inputs["particles"], np.float32)
    for c in range(NCORES):
        out[1:, c * BL:(c + 1) * BL, :] = \
            np.asarray(res.results[c]["traj"]).reshape(NB, BL, D)
    return out, res


def kernel(**inputs):
    return run(inputs)[0]


# revision 21
# speedup vs baseline: 1.1525x; 1.0029x over previous
"""Bass/Tile kernel for nn_CMCD (annealed Langevin sampler with SVGD repulsion).

SPMD over 8 cores, data-parallel over the particle batch (64 rows/core).

Structure (v2):
- Host precomputes all input-only transforms: time-embedding table, betas,
  weight layouts/casts, noise prescale, and the step-0 particle tiles
  (so step 0 needs no collective).
- A tiny warm-up AllGather fires at t~0 to absorb collective mesh-init /
  core-start skew while weights stream in.
- Per step s>=1: AllGather of x_s posted at the end of step s-1; the
  score net + mixture-gradient run in its shadow; repulsion from the
  gathered particles; fused update.
- Activation-table discipline: steady-state act functions are only
  {Exp, Gelu, Identity, Square} ordered as [exp-block][gelu-block] per
  step -> 2 table loads/step.
- Bandwidth (SVGD median heuristic) replaced by a calibrated sqrt-free
  estimator computed from mean/var of d2 over a 128x512 subsample, one
  step stale (h_s = h(x_{s-1})); step 0/1 bandwidth comes from the host.
  Validated end-to-end at rel err ~2e-5 vs the jax reference.
"""
import numpy as np
from contextlib import ExitStack

import concourse.bass as bass
import concourse.bacc as bacc
import concourse.tile as tile
from concourse import mybir
from concourse.masks import make_identity

D, C, NB, NH, M = 64, 512, 8, 3, 8
B = 512
NCORES = 8
BL = B // NCORES  # 64
KB = C // 128     # 4 channel blocks
LOGN = float(np.log(B))
RSUB = 128 * B    # subsample count for the bandwidth (rows 0..127)
AGW = BL * D + BL  # flat AllGather payload words per core
NJUNK = 400
EPS_A = 2.0        # total d2 shift (bf16-safety); corrected exactly
F32 = mybir.dt.float32
BF16 = mybir.dt.bfloat16
AF = mybir.ActivationFunctionType
ALU = mybir.AluOpType
GELU = AF.Gelu_apprx_tanh


def build_nc(compile=True):
    nc = bacc.Bacc("TRN2", target_bir_lowering=False, debug=False,
                   num_devices=NCORES)

    t = {}
    def din(name, shape, dtype):
        t[name] = nc.dram_tensor(name, shape, dtype, kind="ExternalInput")

    # ---- per-core state inputs ----
    din("x0_loc", [BL, D], F32)
    din("xT0_loc", [D, BL], F32)
    din("xT0_locN2", [D, BL], BF16)
    din("x2locn2_0", [1, BL], BF16)
    din("xall0", [D, B], BF16)        # -2 * x0^T, all particles
    din("x2rowN2_0", [1, B], BF16)    # -2*(|x0_j|^2 + 1), all particles
    din("noises_s", [BL, NB, D], F32)  # pre-scaled by sqrt(2 dt)
    # ---- weights / tables (host-prepped) ----
    din("inWs_bf", [D, C], BF16)       # -0.5 * in_W
    din("te_bf", [1, NB * C], BF16)    # te_s + in_b, flat row
    din("hW_bf", [128, NH * KB * C], BF16)
    din("hb_bf", [1, NH * C], BF16)
    din("outWs_bf", [128, KB * D], BF16)  # dt * out_W
    din("outbs_bf", [1, D], BF16)         # dt * out_b
    din("means", [M, D], F32)
    din("meansT", [D, M], F32)
    din("negmu2", [1, M], F32)
    din("dtb8", [M, NB], F32)          # col s = -dt*beta_s
    din("row4", [1, 4], F32)           # [.5*logn, A*logn, -.05*dt*logn, .1*dt*logn]
    din("bcast0_row", [1, 4], F32)     # row4 / hL(x0)
    din("omd_col", [128, 1], F32)      # 1 - dt

    traj_d = nc.dram_tensor("traj", [NB, BL, D], F32, kind="ExternalOutput")
    t["traj_d"] = traj_d

    # collective bounce buffers: steps 1..NB-1, plus a warm-up dummy
    t["agin"] = [None] + [nc.dram_tensor(f"agin{s}", [AGW], BF16)
                          for s in range(1, NB)]
    t["agout"] = [None] + [nc.dram_tensor(f"agout{s}", [NCORES, AGW], BF16,
                                          addr_space="Shared")
                           for s in range(1, NB)]
    t["dd_in"] = nc.dram_tensor("dd_in", [64], BF16)
    t["dd_out"] = nc.dram_tensor("dd_out", [NCORES, 64], BF16,
                                 addr_space="Shared")

    with tile.TileContext(nc) as tc, ExitStack() as ctx:
        _body(ctx, tc, nc, t)
    if compile:
        nc.compile()
    return nc


def _body(ctx, tc, nc, t):
    traj_d, agin, agout = t["traj_d"], t["agin"], t["agout"]

    const = ctx.enter_context(tc.tile_pool(name="const", bufs=1))
    wpool = ctx.enter_context(tc.tile_pool(name="wpool", bufs=1))
    sb2 = ctx.enter_context(tc.tile_pool(name="sb2", bufs=2))
    sb3 = ctx.enter_context(tc.tile_pool(name="sb3", bufs=3))
    scratch = ctx.enter_context(tc.tile_pool(name="scratch", bufs=2))
    ps_small = ctx.enter_context(tc.tile_pool(name="ps_small", bufs=2, space="PSUM"))
    ps_d2f = ctx.enter_context(tc.tile_pool(name="ps_d2f", bufs=1, space="PSUM"))
    ps_d2l = ctx.enter_context(tc.tile_pool(name="ps_d2l", bufs=1, space="PSUM"))
    ps_u = ctx.enter_context(tc.tile_pool(name="ps_u", bufs=1, space="PSUM"))
    ps_cmp = ctx.enter_context(tc.tile_pool(name="ps_cmp", bufs=1, space="PSUM"))
    ps_net = ctx.enter_context(tc.tile_pool(name="ps_net", bufs=2, space="PSUM"))

    # ---------------- warm-up collective: very first instruction ----------------
    nc.gpsimd.collective_compute(
        "AllGather", ALU.bypass, replica_groups=[list(range(NCORES))],
        ins=[t["dd_in"].ap().opt()], outs=[t["dd_out"].ap().opt()])

    # ---------------- input DMAs (3 queues, ordered by first use) ----------------
    # queue A (sync): step-0 particle tiles + noises
    x0_loc = wpool.tile([BL, D], F32)
    nc.sync.dma_start(out=x0_loc, in_=t["x0_loc"][:, :])
    xT0_loc = wpool.tile([D, BL], F32)
    nc.sync.dma_start(out=xT0_loc, in_=t["xT0_loc"][:, :])
    xT0_locN2 = wpool.tile([D, BL], BF16)
    nc.sync.dma_start(out=xT0_locN2, in_=t["xT0_locN2"][:, :])
    x2locn2_0 = wpool.tile([1, BL], BF16)
    nc.sync.dma_start(out=x2locn2_0, in_=t["x2locn2_0"][:, :])
    xall0 = wpool.tile([D, NCORES, BL], BF16)
    nc.sync.dma_start(out=xall0, in_=t["xall0"].ap().rearrange(
        "d (c b) -> d c b", c=NCORES))
    x2rowN2_0 = wpool.tile([1, B], BF16)
    nc.sync.dma_start(out=x2rowN2_0, in_=t["x2rowN2_0"][:, :])
    noise_sb = wpool.tile([BL, NB, D], F32)
    nc.sync.dma_start(out=noise_sb, in_=t["noises_s"][:, :, :])
    # queue A continues: second half of hW
    hW_sb = wpool.tile([128, NH, KB, KB, 128], BF16)
    # queue B (scalar): small weights in use order
    inWs_bf = wpool.tile([D, C], BF16)
    nc.scalar.dma_start(out=inWs_bf, in_=t["inWs_bf"][:, :])
    te_bf = wpool.tile([1, NB * C], BF16)
    nc.scalar.dma_start(out=te_bf[0:1, 0:NB * C // 2],
                        in_=t["te_bf"][0:1, 0:NB * C // 2])
    nc.sync.dma_start(out=te_bf[0:1, NB * C // 2:],
                      in_=t["te_bf"][0:1, NB * C // 2:])
    meansT_sb = wpool.tile([D, M], F32)
    nc.scalar.dma_start(out=meansT_sb, in_=t["meansT"][:, :])
    negmu2_row = wpool.tile([1, M], F32)
    nc.scalar.dma_start(out=negmu2_row, in_=t["negmu2"][:, :])
    means_sb = wpool.tile([M, D], F32)
    nc.scalar.dma_start(out=means_sb, in_=t["means"][:, :])
    dtb8_sb = wpool.tile([M, NB], F32)
    nc.scalar.dma_start(out=dtb8_sb, in_=t["dtb8"][:, :])
    row4_sb = wpool.tile([1, 4], F32)
    nc.scalar.dma_start(out=row4_sb, in_=t["row4"][:, :])
    bcast0_row = wpool.tile([1, 4], F32)
    nc.scalar.dma_start(out=bcast0_row, in_=t["bcast0_row"][:, :])
    omd_col = wpool.tile([128, 1], F32)
    nc.scalar.dma_start(out=omd_col, in_=t["omd_col"][:, :])
    hb_sb = wpool.tile([1, NH * C], BF16)
    nc.scalar.dma_start(out=hb_sb, in_=t["hb_bf"][:, :])
    outWs_sb = wpool.tile([128, KB, D], BF16)
    nc.scalar.dma_start(out=outWs_sb, in_=t["outWs_bf"].ap().rearrange(
        "p (k d) -> p k d", k=KB))
    outbs_row = wpool.tile([1, D], BF16)
    nc.scalar.dma_start(out=outbs_row, in_=t["outbs_bf"][:, :])
    # hW split across queues A and B (~750KB each)
    hWr = t["hW_bf"].ap().rearrange("p (l a b q) -> p l a b q", l=NH, a=KB,
                                    b=KB)
    nc.scalar.dma_start(out=hW_sb[:, 0:2, :, :, :], in_=hWr[:, 0:2, :, :, :])
    nc.sync.dma_start(out=hW_sb[:, 2:NH, :, :, :], in_=hWr[:, 2:NH, :, :, :])

    # ---------------- constants ----------------
    ident = const.tile([128, 128], F32)
    make_identity(nc, ident)
    ident_bf = const.tile([128, 128], BF16)
    nc.vector.tensor_copy(ident_bf, ident)
    ones_col = const.tile([128, 1], F32)
    nc.vector.memset(ones_col, 1.0)
    ones_row = const.tile([1, 128], F32)
    nc.vector.memset(ones_row, 1.0)
    ones_row_bf = const.tile([1, 128], BF16)
    nc.vector.memset(ones_row_bf, 1.0)
    ones_col_bf = const.tile([128, 1], BF16)
    nc.vector.memset(ones_col_bf, 1.0)

    # broadcast bcast0_row -> [128, 4] (used by steps 0 and 1)
    bc0_ps = ps_small.tile([128, 4], F32, tag="sm", name="bc0_ps")
    nc.tensor.matmul(bc0_ps, lhsT=ones_row, rhs=bcast0_row, start=True, stop=True)
    bc0 = const.tile([128, 4], F32)
    nc.vector.tensor_copy(bc0, bc0_ps)

    # ---------------- per-step state handles ----------------
    x_loc = x0_loc
    xT_loc = xT0_loc
    xT_locN2 = xT0_locN2
    x2locn2 = x2locn2_0
    bc_next = bc0  # bandwidth broadcast for the *next* issued step

    for s in range(NB):
        bc = bc_next
        # ---- gathered particle tiles ----
        if s == 0:
            xall = xall0
            x2rowN2 = x2rowN2_0
        else:
            xall = sb2.tile([D, NCORES, BL], BF16, tag="xall", name=f"xall{s}")
            for half, eng in ((0, nc.sync), (1, nc.scalar)):
                eng.dma_start(
                    out=xall[:, half * 4:(half + 1) * 4, :],
                    in_=bass.AP(tensor=agout[s].ap().tensor,
                                offset=half * 4 * AGW,
                                ap=[[BL, D], [AGW, 4], [1, BL]]))
            x2rowN2 = sb3.tile([1, B], BF16, tag="x2rowN2", name=f"x2r{s}")
            nc.sync.dma_start(
                out=x2rowN2.rearrange("o (c b) -> o c b", c=NCORES),
                in_=bass.AP(tensor=agout[s].ap().tensor, offset=BL * D,
                            ap=[[0, 1], [AGW, NCORES], [1, BL]]))

        # ================= TENSOR (+ matching act/vector) =================
        # ---- mixture-component logits (feeds softmax exp, first act op) ----
        comp_ps = ps_cmp.tile([BL, M], F32, tag="cmp", name=f"comp{s}")
        nc.tensor.matmul(comp_ps, lhsT=xT_loc, rhs=meansT_sb, start=True, stop=False)
        nc.tensor.matmul(comp_ps, lhsT=ones_row[0:1, 0:BL], rhs=negmu2_row,
                         start=False, stop=True)
        negmax = sb3.tile([BL, 1], F32, tag="negmax", name=f"negmax{s}")
        nc.vector.tensor_reduce(negmax, comp_ps, axis=mybir.AxisListType.X,
                                op=ALU.max, negate=True)
        # softmax exps via Tanh (resident in the gelu table):
        # e^-w = (1-tanh(w/2))/(1+tanh(w/2)), w = -(comp+negmax) >= 0
        halfmax = sb3.tile([BL, 1], F32, tag="halfmax", name=f"halfmax{s}")
        nc.vector.tensor_scalar(halfmax, negmax, -0.5, None, ALU.mult)
        tsm = sb3.tile([BL, M], F32, tag="tsm", name=f"tsm{s}")
        nc.scalar.activation(tsm, comp_ps, AF.Tanh, bias=halfmax, scale=-0.5)
        numt = sb3.tile([BL, M], F32, tag="numt", name=f"numt{s}")
        nc.vector.tensor_scalar(numt, tsm, -1.0, 1.0, ALU.mult, ALU.add)
        dent = sb3.tile([BL, M], F32, tag="dent", name=f"dent{s}")
        nc.vector.tensor_scalar(dent, tsm, 1.0, 1.0, ALU.mult, ALU.add)
        rden = sb3.tile([BL, M], F32, tag="rden", name=f"rden{s}")
        nc.vector.reciprocal(rden, dent)
        w_un = sb3.tile([BL, M], F32, tag="w_un", name=f"w_un{s}")
        nc.vector.tensor_tensor(w_un, numt, rden, ALU.mult)
        sumexp = sb3.tile([BL, 1], F32, tag="sumexp", name=f"sumexp{s}")
        nc.vector.tensor_reduce(sumexp, w_un, axis=mybir.AxisListType.X,
                                op=ALU.add)
        rcp = sb3.tile([BL, 1], F32, tag="rcp", name=f"rcp{s}")
        nc.vector.reciprocal(rcp, sumexp)
        w_n = sb3.tile([BL, M], F32, tag="w_n", name=f"w_n{s}")
        nc.vector.tensor_scalar(w_n, w_un, rcp, None, ALU.mult)

        # ---- score net layer 1: h^T = (x @ in_W)^T + te (gelu) ----
        h_ps = ps_net.tile([128, KB, BL], F32, tag="h_ps", bufs=1, name=f"h_ps{s}")
        for ko in range(KB):
            nc.tensor.matmul(h_ps[:, ko, :],
                             lhsT=inWs_bf[:, 128 * ko:128 * (ko + 1)],
                             rhs=xT_locN2, start=True, stop=False)
            nc.tensor.matmul(h_ps[:, ko, :],
                             lhsT=te_bf[0:1, s * C + 128 * ko:
                                        s * C + 128 * (ko + 1)],
                             rhs=ones_row_bf[0:1, 0:BL], start=False, stop=True)
        h_sb = sb2.tile([128, KB, BL], BF16, tag="h0", name=f"h0_{s}")
        # act #2: Gelu (table load; net has slack vs the collective path)
        nc.scalar.activation(h_sb.rearrange("p k b -> p (k b)"),
                             h_ps.rearrange("p k b -> p (k b)"), GELU)

        # ---- softmax tail on tensor (w^T, scaled by -dt*beta_s) ----
        wT_ps = ps_small.tile([M, BL], F32, tag="sm", name=f"wT{s}")
        nc.tensor.transpose(wT_ps, w_n, ident[0:BL, 0:BL])
        wTs_sb = sb3.tile([M, BL], F32, tag="wTs", name=f"wTs{s}")
        nc.vector.tensor_scalar(wTs_sb, wT_ps, dtb8_sb[0:M, s:s + 1], None, ALU.mult)

        # ---- hidden layers, fully transposed flow: h^T -> h^T ----
        # hT[co,b] = gelu(sum_ci W[ci,co]^T hT[ci,b] + hb[co]); no casts,
        # no transposes between layers.
        for l in range(NH):
            hT_ps = ps_net.tile([128, KB, BL], F32, tag="h_ps", bufs=1,
                                name=f"hT{s}_{l}")
            for ko in range(KB):
                for ki in range(KB):
                    nc.tensor.matmul(hT_ps[:, ko, :],
                                     lhsT=hW_sb[:, l, ki, ko, :],
                                     rhs=h_sb[:, ki, :],
                                     start=(ki == 0), stop=False)
                nc.tensor.matmul(hT_ps[:, ko, :],
                                 lhsT=hb_sb[0:1, l * C + 128 * ko:
                                            l * C + 128 * (ko + 1)],
                                 rhs=ones_row_bf[0:1, 0:BL],
                                 start=False, stop=True)
            hn_sb = sb2.tile([128, KB, BL], BF16, tag=f"h{l + 1}",
                             name=f"hn_sb{s}_{l}")
            nc.scalar.activation(hn_sb.rearrange("p k b -> p (k b)"),
                                 hT_ps.rearrange("p k b -> p (k b)"), GELU)
            h_sb = hn_sb

        # ---- pairwise d2, local columns: psum = -2*(d2+A) ----
        d2l_ps = ps_d2l.tile([128, KB, BL], F32, tag="d2l", name=f"d2l{s}")
        for k in range(KB):
            nc.tensor.matmul(d2l_ps[:, k, :], lhsT=xall[:, 2 * k:2 * k + 2, :],
                             rhs=xT_locN2, start=True, stop=False)
            nc.tensor.matmul(d2l_ps[:, k, :], lhsT=ones_row_bf[0:1, 0:128],
                             rhs=x2locn2, start=False, stop=False)
            nc.tensor.matmul(d2l_ps[:, k, :],
                             lhsT=x2rowN2[0:1, 128 * k:128 * (k + 1)],
                             rhs=ones_row_bf[0:1, 0:BL], start=False, stop=True)

        # ---- repulsion kernel: kt = exp(-d2/h) ----
        kt_sb = sb2.tile([128, KB, BL], BF16, tag="kt", name=f"kt{s}")
        # act: Exp (table load #2; hides under the collective/d2l window)
        nc.scalar.activation(kt_sb.rearrange("p k b -> p (k b)"),
                             d2l_ps.rearrange("p k b -> p (k b)"), AF.Exp,
                             bias=bc[:, 1:2], scale=bc[:, 0:1])

        # ---- x rows scaled by c_h: xfe = c_h * x  (from -2x^T blocks) ----
        xft_ps = ps_net.tile([128, KB, BL], BF16, tag="h_ps", bufs=1,
                             name=f"xft{s}")
        for k in range(KB):
            nc.tensor.transpose(xft_ps[:, k, :], xall[:, 2 * k:2 * k + 2, :],
                                ident_bf[0:D, 0:D])
        xfe = sb2.tile([128, KB, BL], BF16, tag="xfe", name=f"xfe{s}")
        nc.vector.tensor_scalar(xfe.rearrange("p k b -> p (k b)"),
                                xft_ps.rearrange("p k b -> p (k b)"),
                                bc[:, 2:3], None, ALU.mult)

        # ---- U = dt*score + dt*out_b - dt*beta*(w@means) ----
        u_ps = ps_u.tile([BL, D], F32, tag="u", name=f"u{s}")
        for ki in range(KB):
            nc.tensor.matmul(u_ps, lhsT=h_sb[:, ki, :], rhs=outWs_sb[:, ki, :],
                             start=(ki == 0), stop=False)
        nc.tensor.matmul(u_ps, lhsT=ones_row_bf[0:1, 0:BL], rhs=outbs_row,
                         start=False, stop=False)
        nc.tensor.matmul(u_ps, lhsT=wTs_sb, rhs=means_sb, start=False, stop=False)

        # ---- kxr = c_h * K@x ; rch = c_h * r ----
        for k in range(KB):
            nc.tensor.matmul(u_ps, lhsT=kt_sb[:, k, :], rhs=xfe[:, k, :],
                             start=False, stop=(k == KB - 1))
        chcol_bf = sb3.tile([128, 1], BF16, tag="chcol", name=f"chcol{s}")
        nc.vector.tensor_copy(chcol_bf, bc[:, 3:4])
        rch_ps = ps_small.tile([BL, 1], F32, tag="sm", name=f"rch{s}")
        for k in range(KB):
            nc.tensor.matmul(rch_ps, lhsT=kt_sb[:, k, :], rhs=chcol_bf,
                             start=(k == 0), stop=(k == KB - 1))

        # ---- update: new = x*(1-dt+c_h*r) + noise - U - c_h*K@x ----
        alpha = sb3.tile([BL, 1], F32, tag="alpha", name=f"alpha{s}")
        nc.vector.tensor_tensor(alpha, rch_ps, omd_col[0:BL, 0:1], ALU.add)
        t1 = sb3.tile([BL, D], F32, tag="t1", name=f"t1_{s}")
        nc.vector.tensor_scalar(t1, x_loc, alpha, None, ALU.mult)
        t2 = sb3.tile([BL, D], F32, tag="t2", name=f"t2_{s}")
        nc.vector.tensor_tensor(t2, t1, noise_sb[:, s, :], ALU.add)
        new_x = sb2.tile([BL, D], F32, tag="x_loc", name=f"x{s + 1}")
        nc.vector.tensor_tensor(new_x, t2, u_ps, ALU.subtract)
        nc.scalar.dma_start(out=traj_d[s], in_=new_x)

        # ---- stage + post AllGather for step s+1 (one DMA: [-2x^T | x2]) ----
        if s + 1 < NB:
            nxT_ps = ps_small.tile([D, BL], F32, tag="sm", name=f"nxT{s + 1}")
            nc.tensor.transpose(nxT_ps, new_x, ident[0:BL, 0:BL])
            stg = sb2.tile([D + 1, BL], BF16, tag="stg", name=f"stg{s + 1}")
            nc.vector.tensor_scalar(stg[0:D, :], nxT_ps, -2.0, None, ALU.mult)
            nxT_loc = sb2.tile([D, BL], F32, tag="xT_loc", name=f"xT{s + 1}")
            nc.vector.tensor_copy(nxT_loc, nxT_ps)
            sqnT = scratch.tile([D, BL], F32, tag="sqnT", name=f"sqnT{s + 1}")
            nc.vector.tensor_tensor(sqnT, nxT_ps, nxT_loc, ALU.mult)
            x2l_ps = ps_small.tile([1, BL], F32, tag="sm", name=f"x2l{s + 1}")
            nc.tensor.matmul(x2l_ps, lhsT=ones_col[0:D, 0:1], rhs=sqnT,
                             start=True, stop=True)
            nc.vector.tensor_scalar(stg[D:D + 1, :], x2l_ps, -2.0, -2.0,
                                    ALU.mult, ALU.add)
            nc.sync.dma_start(
                out=agin[s + 1].ap()[0:AGW].rearrange("(p b) -> p b", p=D + 1),
                in_=stg)
            nc.gpsimd.collective_compute(
                "AllGather", ALU.bypass, replica_groups=[list(range(NCORES))],
                ins=[agin[s + 1].ap().opt()], outs=[agout[s + 1].ap().opt()])
            nx2locn2 = sb2.tile([1, BL], BF16, tag="x2locn2", name=f"x2n2_{s + 1}")
            nc.vector.tensor_copy(nx2locn2, stg[D:D + 1, :])
            x_loc, xT_loc = new_x, nxT_loc
            xT_locN2 = stg[0:D, :]
            x2locn2 = nx2locn2

        # ---- 2-step-stale bandwidth: process the PREVIOUS step's gather
        # (bc for step s+1 = h(x_{s-1})); issued after the stage/trigger so
        # its tensor/act work lands in this step's idle windows.
        if 1 <= s <= NB - 2:
            d2f_ps = ps_d2f.tile([128, B], F32, tag="d2f", name=f"d2f{s}")
            nc.tensor.matmul(d2f_ps, lhsT=xall_prev[:, 0:2, :],
                             rhs=xall_prev.rearrange("d c b -> d (c b)"),
                             start=True, stop=False)
            nc.tensor.matmul(d2f_ps, lhsT=ones_row_bf[0:1, 0:128],
                             rhs=x2row_prev,
                             start=False, stop=True)
            x2c_ps = ps_small.tile([128, 1], F32, tag="sm", name=f"x2c{s}")
            nc.tensor.matmul(x2c_ps, lhsT=x2row_prev[0:1, 0:128],
                             rhs=ones_col_bf[0:1, 0:1], start=True, stop=True)
            x2colP = sb3.tile([128, 1], F32, tag="x2colP", name=f"x2colP{s}")
            nc.vector.tensor_scalar(x2colP, x2c_ps, -0.5, None, ALU.mult)
            dsums = sb3.tile([128, 2], F32, tag="dsums", name=f"dsums{s}")
            zscr = scratch.tile([128, B], BF16, tag="zscr", name=f"zscr{s}")
            zscr2 = scratch.tile([128, B], BF16, tag="zscr2", name=f"zscr2{s}")
            # act (exp table, Identity+Square are in every table): z, z^2 sums
            nc.scalar.activation(zscr, d2f_ps, AF.Identity, bias=x2colP,
                                 scale=-0.5, accum_out=dsums[:, 0:1])
            nc.scalar.activation(zscr2, d2f_ps, AF.Square, bias=x2colP,
                                 scale=-0.5, accum_out=dsums[:, 1:2])
            sum12_ps = ps_small.tile([1, 2], F32, tag="sm", name=f"sum12{s}")
            nc.tensor.matmul(sum12_ps, lhsT=ones_col, rhs=dsums, start=True,
                             stop=True)
            # scalar chain: m=(S0/RS), m2=(S1/RS); q=(m2/m^2-1)/8
            # P = m*(1-q)^2 ~ E[sqrt(z)]^2 ; hL = P - A + A^2/(4P); bc=row4/hL
            mrow = sb3.tile([1, 2], F32, tag="mrow", name=f"mrow{s}")
            nc.vector.tensor_scalar(mrow, sum12_ps, 1.0 / float(RSUB), None,
                                    ALU.mult)
            msq = sb3.tile([1, 1], F32, tag="msq", name=f"msq{s}")
            nc.gpsimd.tensor_tensor(msq, mrow[0:1, 0:1], mrow[0:1, 0:1], ALU.mult)
            rmsq = sb3.tile([1, 1], F32, tag="rmsq", name=f"rmsq{s}")
            nc.vector.reciprocal(rmsq, msq)
            t2m = sb3.tile([1, 1], F32, tag="t2m", name=f"t2m{s}")
            nc.gpsimd.tensor_tensor(t2m, mrow[0:1, 1:2], rmsq, ALU.mult)
            uq = sb3.tile([1, 1], F32, tag="uq", name=f"uq{s}")
            nc.gpsimd.tensor_scalar(uq, t2m, -0.125, 1.125, ALU.mult, ALU.add)
            uq2 = sb3.tile([1, 1], F32, tag="uq2", name=f"uq2{s}")
            nc.gpsimd.tensor_tensor(uq2, uq, uq, ALU.mult)
            Pm = sb3.tile([1, 1], F32, tag="Pm", name=f"Pm{s}")
            nc.gpsimd.tensor_tensor(Pm, uq2, mrow[0:1, 0:1], ALU.mult)
            rP = sb3.tile([1, 1], F32, tag="rP", name=f"rP{s}")
            nc.vector.reciprocal(rP, Pm)
            z1 = sb3.tile([1, 1], F32, tag="z1", name=f"z1{s}")
            nc.gpsimd.tensor_scalar(z1, rP, EPS_A * EPS_A / 4.0, -EPS_A,
                                    ALU.mult, ALU.add)
            hL = sb3.tile([1, 1], F32, tag="hL", name=f"hL{s}")
            nc.gpsimd.tensor_tensor(hL, z1, Pm, ALU.add)
            rhL = sb3.tile([1, 1], F32, tag="rhL", name=f"rhL{s}")
            nc.vector.reciprocal(rhL, hL)
            rep4 = sb3.tile([1, 4], F32, tag="rep4", name=f"rep4{s}")
            nc.gpsimd.tensor_scalar(rep4, row4_sb, rhL, None, ALU.mult)
            nbc_ps = ps_small.tile([128, 4], F32, tag="sm", name=f"nbc{s}")
            nc.tensor.matmul(nbc_ps, lhsT=ones_row, rhs=rep4, start=True,
                             stop=True)
            nbc = sb2.tile([128, 4], F32, tag="bc", name=f"bc{s + 2}")
            nc.vector.tensor_copy(nbc, nbc_ps)
            bc_next = nbc
        # (s == 0 keeps bc_next = bc0 for step 1; s == NB-1 ends the loop)
        xall_prev, x2row_prev = xall, x2rowN2


# ======================================================================
# Host-side wrapper: prep + shard inputs, run SPMD on 8 cores, gather.
# ======================================================================
_CACHE = {}


def _get_nc():
    if "nc" not in _CACHE:
        _CACHE["nc"] = build_nc()
    return _CACHE["nc"]


def _np_gelu(x):
    return 0.5 * x * (1.0 + np.tanh(np.sqrt(2.0 / np.pi)
                                    * (x + 0.044715 * x ** 3)))


def _prep(inputs):
    """Host-side input-only transforms shared by all cores."""
    import ml_dtypes
    bf16 = ml_dtypes.bfloat16
    f32 = np.float32
    g = {}
    dt = float(np.asarray(inputs["eps"], np.float64)[0])
    x0 = np.asarray(inputs["particles"], np.float64)          # [B, D]

    # betas
    sig = 1.0 / (1.0 + np.exp(-np.asarray(inputs["grid_t"], np.float64)))
    betas = np.concatenate([[0.0], np.cumsum(sig)]) / sig.sum()

    # time-embedding table: te_s + in_b  [NB, C]
    coeff = np.linspace(0.1, 100.0, C, dtype=np.float64)[None, :]
    phase = np.asarray(inputs["phase"], np.float64)
    tW1 = np.asarray(inputs["t_W1"], np.float64)
    tW2 = np.asarray(inputs["t_W2"], np.float64)
    TE = np.zeros((NB, C))
    for s in range(NB):
        emb = coeff * s + phase
        temb = np.concatenate([np.sin(emb), np.cos(emb)], -1)
        te = _np_gelu(temb @ tW1 + np.asarray(inputs["t_b1"], np.float64)) \
            @ tW2 + np.asarray(inputs["t_b2"], np.float64)
        TE[s] = te + np.asarray(inputs["in_b"], np.float64)
    g["te_bf"] = TE.astype(f32).astype(bf16).reshape(1, NB * C)

    g["inWs_bf"] = (-0.5 * np.asarray(inputs["in_W"], f32)).astype(bf16)
    hW = np.asarray(inputs["h_W"], f32)                        # [NH, C, C]
    g["hW_bf"] = np.ascontiguousarray(
        hW.reshape(NH, KB, 128, KB, 128).transpose(2, 0, 1, 3, 4)
        .reshape(128, -1)).astype(bf16)
    g["hb_bf"] = np.asarray(inputs["h_b"], f32).astype(bf16).reshape(1, NH * C)
    outW = np.asarray(inputs["out_W"], f32)                    # [C, D]
    g["outWs_bf"] = np.ascontiguousarray(
        (dt * outW).reshape(KB, 128, D).transpose(1, 0, 2).reshape(128, -1)
    ).astype(bf16)
    g["outbs_bf"] = (dt * np.asarray(inputs["out_b"], f32)[None, :]).astype(bf16)

    means = np.asarray(inputs["target_means"], f32)
    g["means"] = means
    g["meansT"] = np.ascontiguousarray(means.T)
    g["negmu2"] = (-0.5 * (means.astype(np.float64) ** 2).sum(-1)[None, :]
                   ).astype(f32)
    g["dtb8"] = np.tile((-dt * betas[:NB]).astype(f32)[None, :], (M, 1))
    row4 = np.array([[0.5 * LOGN, EPS_A * LOGN, -0.05 * dt * LOGN,
                      0.1 * dt * LOGN]], np.float64)
    g["row4"] = row4.astype(f32)

    # host bandwidth for steps 0 and 1: hL = h*logn from x0 subsample
    f = x0[:128, None, :] - x0[None, :, :]
    z = (f * f).sum(-1) + EPS_A
    m, m2 = z.mean(), (z * z).mean()
    q = (m2 - m * m) / (8.0 * m * m)
    P = m * (1.0 - q) ** 2
    hL0 = P - EPS_A + EPS_A * EPS_A / (4.0 * P)
    g["bcast0_row"] = (row4 / hL0).astype(f32)
    g["omd_col"] = np.full((128, 1), 1.0 - dt, f32)

    # full-particle tiles for step 0
    x0f = x0.astype(f32)
    g["xall0"] = np.ascontiguousarray(-2.0 * x0f.T).astype(bf16)
    x2 = (x0f * x0f).sum(-1) + 1.0
    g["x2rowN2_0"] = (-2.0 * x2[None, :]).astype(bf16)

    # noise, pre-scaled, [B, NB, D]
    noi = np.asarray(inputs["noises"], f32) * np.float32(np.sqrt(2.0 * dt))
    g["noises_all"] = np.ascontiguousarray(noi.transpose(1, 0, 2))
    g["x0f"] = x0f
    return g


def _shard(g, c):
    import ml_dtypes
    bf16 = ml_dtypes.bfloat16
    sl = slice(c * BL, (c + 1) * BL)
    m = {k: g[k] for k in ["inWs_bf", "te_bf", "hW_bf", "hb_bf", "outWs_bf",
                           "outbs_bf", "means", "meansT", "negmu2", "dtb8",
                           "row4", "bcast0_row", "omd_col", "xall0",
                           "x2rowN2_0"]}
    x0l = np.ascontiguousarray(g["x0f"][sl])
    m["x0_loc"] = x0l
    m["xT0_loc"] = np.ascontiguousarray(x0l.T)
    m["xT0_locN2"] = np.ascontiguousarray(-2.0 * x0l.T).astype(bf16)
    x2 = (x0l * x0l).sum(-1) + 1.0
    m["x2locn2_0"] = (-2.0 * x2[None, :]).astype(bf16)
    m["noises_s"] = np.ascontiguousarray(g["noises_all"][sl])
    return m


def run(inputs, trace=False, trace_cores=None):
    from concourse.bass_utils import run_bass_kernel_spmd
    nc = _get_nc()
    g = _prep(inputs)
    in_maps = [_shard(g, c) for c in range(NCORES)]
    res = run_bass_kernel_spmd(nc, in_maps, core_ids=list(range(NCORES)),
                               trace=trace, trace_cores=trace_cores)
    out = np.zeros((NB + 1, B, D), np.float32)
    out[0] = np.asarray(inputs["particles"], np.float32)
    for c in range(NCORES):
        out[1:, c * BL:(c + 1) * BL, :] = \
            np.asarray(res.results[c]["traj"]).reshape(NB, BL, D)
    return out, res


def kernel(**inputs):
    return run(inputs)[0]


# revision 22
# speedup vs baseline: 1.2436x; 1.0791x over previous
"""Bass/Tile kernel for nn_CMCD (annealed Langevin sampler with SVGD repulsion).

SPMD over 8 cores, data-parallel over the particle batch (64 rows/core).

Structure (v2):
- Host precomputes all input-only transforms: time-embedding table, betas,
  weight layouts/casts, noise prescale, and the step-0 particle tiles
  (so step 0 needs no collective).
- A tiny warm-up AllGather fires at t~0 to absorb collective mesh-init /
  core-start skew while weights stream in.
- Per step s>=1: AllGather of x_s posted at the end of step s-1; the
  score net + mixture-gradient run in its shadow; repulsion from the
  gathered particles; fused update.
- Activation-table discipline: steady-state act functions are only
  {Exp, Gelu, Identity, Square} ordered as [exp-block][gelu-block] per
  step -> 2 table loads/step.
- Bandwidth (SVGD median heuristic) replaced by a calibrated sqrt-free
  estimator computed from mean/var of d2 over a 128x512 subsample, one
  step stale (h_s = h(x_{s-1})); step 0/1 bandwidth comes from the host.
  Validated end-to-end at rel err ~2e-5 vs the jax reference.
"""
import numpy as np
from contextlib import ExitStack

import concourse.bass as bass
import concourse.bacc as bacc
import concourse.tile as tile
from concourse import mybir
from concourse.masks import make_identity

D, C, NB, NH, M = 64, 512, 8, 3, 8
B = 512
NCORES = 8
BL = B // NCORES  # 64
KB = C // 128     # 4 channel blocks
LOGN = float(np.log(B))
RSUB = 128 * B    # subsample count for the bandwidth (rows 0..127)
AGW = BL * D  # flat AllGather payload words per core (x2 recomputed receiver-side)
NJUNK = 400
EPS_A = 2.0        # total d2 shift (bf16-safety); corrected exactly
F32 = mybir.dt.float32
BF16 = mybir.dt.bfloat16
AF = mybir.ActivationFunctionType
ALU = mybir.AluOpType
GELU = AF.Gelu_apprx_tanh


def build_nc(compile=True):
    nc = bacc.Bacc("TRN2", target_bir_lowering=False, debug=False,
                   num_devices=NCORES)

    t = {}
    def din(name, shape, dtype):
        t[name] = nc.dram_tensor(name, shape, dtype, kind="ExternalInput")

    # ---- per-core state inputs ----
    din("x0_loc", [BL, D], F32)
    din("xT0_loc", [D, BL], F32)
    din("xT0_locN2", [D, BL], BF16)
    din("x2locn2_0", [1, BL], BF16)
    din("xall0", [D, B], BF16)        # -2 * x0^T, all particles
    din("x2rowN2_0", [1, B], BF16)    # -2*(|x0_j|^2 + 1), all particles
    din("noises_s", [BL, NB, D], F32)  # pre-scaled by sqrt(2 dt)
    # ---- weights / tables (host-prepped) ----
    din("inWs_bf", [D, C], BF16)       # -0.5 * in_W
    din("te_bf", [1, NB * C], BF16)    # te_s + in_b, flat row
    din("hW_bf", [128, NH * KB * C], BF16)
    din("hb_bf", [1, NH * C], BF16)
    din("outWs_bf", [128, KB * D], BF16)  # dt * out_W
    din("outbs_bf", [1, D], BF16)         # dt * out_b
    din("means", [M, D], F32)
    din("meansT", [D, M], F32)
    din("negmu2", [1, M], F32)
    din("dtb8", [M, NB], F32)          # col s = -dt*beta_s
    din("row4", [1, 4], F32)           # [.5*logn, A*logn, -.05*dt*logn, .1*dt*logn]
    din("bcast0_row", [1, 4], F32)     # row4 / hL(x0)
    din("omd_col", [128, 1], F32)      # 1 - dt

    traj_d = nc.dram_tensor("traj", [NB, BL, D], F32, kind="ExternalOutput")
    t["traj_d"] = traj_d

    # collective bounce buffers: steps 1..NB-1, plus a warm-up dummy
    t["agin"] = [None] + [nc.dram_tensor(f"agin{s}", [AGW], BF16)
                          for s in range(1, NB)]
    t["agout"] = [None] + [nc.dram_tensor(f"agout{s}", [NCORES, AGW], BF16,
                                          addr_space="Shared")
                           for s in range(1, NB)]
    t["dd_in"] = nc.dram_tensor("dd_in", [64], BF16)
    t["dd_out"] = nc.dram_tensor("dd_out", [NCORES, 64], BF16,
                                 addr_space="Shared")

    with tile.TileContext(nc) as tc, ExitStack() as ctx:
        _body(ctx, tc, nc, t)
    if compile:
        nc.compile()
    return nc


def _body(ctx, tc, nc, t):
    traj_d, agin, agout = t["traj_d"], t["agin"], t["agout"]

    const = ctx.enter_context(tc.tile_pool(name="const", bufs=1))
    wpool = ctx.enter_context(tc.tile_pool(name="wpool", bufs=1))
    sb2 = ctx.enter_context(tc.tile_pool(name="sb2", bufs=2))
    sb3 = ctx.enter_context(tc.tile_pool(name="sb3", bufs=3))
    scratch = ctx.enter_context(tc.tile_pool(name="scratch", bufs=2))
    ps_small = ctx.enter_context(tc.tile_pool(name="ps_small", bufs=2, space="PSUM"))
    ps_d2f = ctx.enter_context(tc.tile_pool(name="ps_d2f", bufs=1, space="PSUM"))
    ps_d2l = ctx.enter_context(tc.tile_pool(name="ps_d2l", bufs=1, space="PSUM"))
    ps_u = ctx.enter_context(tc.tile_pool(name="ps_u", bufs=1, space="PSUM"))
    ps_cmp = ctx.enter_context(tc.tile_pool(name="ps_cmp", bufs=1, space="PSUM"))
    ps_net = ctx.enter_context(tc.tile_pool(name="ps_net", bufs=2, space="PSUM"))

    # ---------------- warm-up collective: very first instruction ----------------
    nc.gpsimd.collective_compute(
        "AllGather", ALU.bypass, replica_groups=[list(range(NCORES))],
        ins=[t["dd_in"].ap().opt()], outs=[t["dd_out"].ap().opt()])

    # ---------------- input DMAs (3 queues, ordered by first use) ----------------
    # queue A (sync): step-0 particle tiles + noises
    x0_loc = wpool.tile([BL, D], F32)
    nc.sync.dma_start(out=x0_loc, in_=t["x0_loc"][:, :])
    xT0_loc = wpool.tile([D, BL], F32)
    nc.sync.dma_start(out=xT0_loc, in_=t["xT0_loc"][:, :])
    xT0_locN2 = wpool.tile([D, BL], BF16)
    nc.sync.dma_start(out=xT0_locN2, in_=t["xT0_locN2"][:, :])
    x2locn2_0 = wpool.tile([1, BL], BF16)
    nc.sync.dma_start(out=x2locn2_0, in_=t["x2locn2_0"][:, :])
    xall0 = wpool.tile([D, NCORES, BL], BF16)
    nc.sync.dma_start(out=xall0, in_=t["xall0"].ap().rearrange(
        "d (c b) -> d c b", c=NCORES))
    x2rowN2_0 = wpool.tile([1, B], BF16)
    nc.sync.dma_start(out=x2rowN2_0, in_=t["x2rowN2_0"][:, :])
    noise_sb = wpool.tile([BL, NB, D], F32)
    nc.sync.dma_start(out=noise_sb, in_=t["noises_s"][:, :, :])
    # queue A continues: second half of hW
    hW_sb = wpool.tile([128, NH, KB, KB, 128], BF16)
    # queue B (scalar): small weights in use order
    inWs_bf = wpool.tile([D, C], BF16)
    nc.scalar.dma_start(out=inWs_bf, in_=t["inWs_bf"][:, :])
    te_bf = wpool.tile([1, NB * C], BF16)
    nc.scalar.dma_start(out=te_bf[0:1, 0:NB * C // 2],
                        in_=t["te_bf"][0:1, 0:NB * C // 2])
    nc.sync.dma_start(out=te_bf[0:1, NB * C // 2:],
                      in_=t["te_bf"][0:1, NB * C // 2:])
    meansT_sb = wpool.tile([D, M], F32)
    nc.scalar.dma_start(out=meansT_sb, in_=t["meansT"][:, :])
    negmu2_row = wpool.tile([1, M], F32)
    nc.scalar.dma_start(out=negmu2_row, in_=t["negmu2"][:, :])
    means_sb = wpool.tile([M, D], F32)
    nc.scalar.dma_start(out=means_sb, in_=t["means"][:, :])
    dtb8_sb = wpool.tile([M, NB], F32)
    nc.scalar.dma_start(out=dtb8_sb, in_=t["dtb8"][:, :])
    row4_sb = wpool.tile([1, 4], F32)
    nc.scalar.dma_start(out=row4_sb, in_=t["row4"][:, :])
    bcast0_row = wpool.tile([1, 4], F32)
    nc.scalar.dma_start(out=bcast0_row, in_=t["bcast0_row"][:, :])
    omd_col = wpool.tile([128, 1], F32)
    nc.scalar.dma_start(out=omd_col, in_=t["omd_col"][:, :])
    hb_sb = wpool.tile([1, NH * C], BF16)
    nc.scalar.dma_start(out=hb_sb, in_=t["hb_bf"][:, :])
    outWs_sb = wpool.tile([128, KB, D], BF16)
    nc.scalar.dma_start(out=outWs_sb, in_=t["outWs_bf"].ap().rearrange(
        "p (k d) -> p k d", k=KB))
    outbs_row = wpool.tile([1, D], BF16)
    nc.scalar.dma_start(out=outbs_row, in_=t["outbs_bf"][:, :])
    # hW split across queues A and B (~750KB each)
    hWr = t["hW_bf"].ap().rearrange("p (l a b q) -> p l a b q", l=NH, a=KB,
                                    b=KB)
    nc.scalar.dma_start(out=hW_sb[:, 0:2, :, :, :], in_=hWr[:, 0:2, :, :, :])
    nc.sync.dma_start(out=hW_sb[:, 2:NH, :, :, :], in_=hWr[:, 2:NH, :, :, :])

    # ---------------- constants ----------------
    ident = const.tile([128, 128], F32)
    make_identity(nc, ident)
    ident_bf = const.tile([128, 128], BF16)
    nc.vector.tensor_copy(ident_bf, ident)
    ones_col = const.tile([128, 1], F32)
    nc.vector.memset(ones_col, 1.0)
    ones_row = const.tile([1, 128], F32)
    nc.vector.memset(ones_row, 1.0)
    ones_row_bf = const.tile([1, 128], BF16)
    nc.vector.memset(ones_row_bf, 1.0)
    ones_col_bf = const.tile([128, 1], BF16)
    nc.vector.memset(ones_col_bf, 1.0)

    # broadcast bcast0_row -> [128, 4] (used by steps 0 and 1)
    bc0_ps = ps_small.tile([128, 4], F32, tag="sm", name="bc0_ps")
    nc.tensor.matmul(bc0_ps, lhsT=ones_row, rhs=bcast0_row, start=True, stop=True)
    bc0 = const.tile([128, 4], F32)
    nc.vector.tensor_copy(bc0, bc0_ps)

    # ---------------- per-step state handles ----------------
    x_loc = x0_loc
    xT_loc = xT0_loc
    xT_locN2 = xT0_locN2
    x2locn2 = x2locn2_0
    bc_next = bc0  # bandwidth broadcast for the *next* issued step

    for s in range(NB):
        bc = bc_next
        # ---- gathered particle tiles ----
        if s == 0:
            xall = xall0
            x2rowN2 = x2rowN2_0
        else:
            xall = sb2.tile([D, NCORES, BL], BF16, tag="xall", name=f"xall{s}")
            for half, eng in ((0, nc.sync), (1, nc.scalar)):
                eng.dma_start(
                    out=xall[:, half * 4:(half + 1) * 4, :],
                    in_=bass.AP(tensor=agout[s].ap().tensor,
                                offset=half * 4 * AGW,
                                ap=[[BL, D], [AGW, 4], [1, BL]]))
            # recompute -2*(|x_j|^2+1) from the gathered -2x^T (gpsimd+PE,
            # off the stage/trigger critical path)
            sq = scratch.tile([D, B], F32, tag="sq", name=f"sq{s}")
            nc.gpsimd.tensor_tensor(sq, xall.rearrange("d c b -> d (c b)"),
                                    xall.rearrange("d c b -> d (c b)"),
                                    ALU.mult)
            x2r_ps = ps_small.tile([1, B], F32, tag="sm", name=f"x2r_ps{s}")
            nc.tensor.matmul(x2r_ps, lhsT=ones_col[0:D, 0:1], rhs=sq,
                             start=True, stop=True)
            x2rowN2 = sb3.tile([1, B], BF16, tag="x2rowN2", name=f"x2r{s}")
            nc.vector.tensor_scalar(x2rowN2, x2r_ps, -0.5, -2.0,
                                    ALU.mult, ALU.add)

        # ================= TENSOR (+ matching act/vector) =================
        # ---- mixture-component logits (feeds softmax exp, first act op) ----
        comp_ps = ps_cmp.tile([BL, M], F32, tag="cmp", name=f"comp{s}")
        nc.tensor.matmul(comp_ps, lhsT=xT_loc, rhs=meansT_sb, start=True, stop=False)
        nc.tensor.matmul(comp_ps, lhsT=ones_row[0:1, 0:BL], rhs=negmu2_row,
                         start=False, stop=True)
        negmax = sb3.tile([BL, 1], F32, tag="negmax", name=f"negmax{s}")
        nc.vector.tensor_reduce(negmax, comp_ps, axis=mybir.AxisListType.X,
                                op=ALU.max, negate=True)
        # softmax exps via Tanh (resident in the gelu table):
        # e^-w = (1-tanh(w/2))/(1+tanh(w/2)), w = -(comp+negmax) >= 0
        halfmax = sb3.tile([BL, 1], F32, tag="halfmax", name=f"halfmax{s}")
        nc.vector.tensor_scalar(halfmax, negmax, -0.5, None, ALU.mult)
        tsm = sb3.tile([BL, M], F32, tag="tsm", name=f"tsm{s}")
        nc.scalar.activation(tsm, comp_ps, AF.Tanh, bias=halfmax, scale=-0.5)
        numt = sb3.tile([BL, M], F32, tag="numt", name=f"numt{s}")
        nc.vector.tensor_scalar(numt, tsm, -1.0, 1.0, ALU.mult, ALU.add)
        dent = sb3.tile([BL, M], F32, tag="dent", name=f"dent{s}")
        nc.vector.tensor_scalar(dent, tsm, 1.0, 1.0, ALU.mult, ALU.add)
        rden = sb3.tile([BL, M], F32, tag="rden", name=f"rden{s}")
        nc.vector.reciprocal(rden, dent)
        w_un = sb3.tile([BL, M], F32, tag="w_un", name=f"w_un{s}")
        nc.vector.tensor_tensor(w_un, numt, rden, ALU.mult)
        sumexp = sb3.tile([BL, 1], F32, tag="sumexp", name=f"sumexp{s}")
        nc.vector.tensor_reduce(sumexp, w_un, axis=mybir.AxisListType.X,
                                op=ALU.add)
        rcp = sb3.tile([BL, 1], F32, tag="rcp", name=f"rcp{s}")
        nc.vector.reciprocal(rcp, sumexp)
        w_n = sb3.tile([BL, M], F32, tag="w_n", name=f"w_n{s}")
        nc.vector.tensor_scalar(w_n, w_un, rcp, None, ALU.mult)

        # ---- score net layer 1: h^T = (x @ in_W)^T + te (gelu) ----
        h_ps = ps_net.tile([128, KB, BL], F32, tag="h_ps", bufs=1, name=f"h_ps{s}")
        for ko in range(KB):
            nc.tensor.matmul(h_ps[:, ko, :],
                             lhsT=inWs_bf[:, 128 * ko:128 * (ko + 1)],
                             rhs=xT_locN2, start=True, stop=False)
            nc.tensor.matmul(h_ps[:, ko, :],
                             lhsT=te_bf[0:1, s * C + 128 * ko:
                                        s * C + 128 * (ko + 1)],
                             rhs=ones_row_bf[0:1, 0:BL], start=False, stop=True)
        h_sb = sb2.tile([128, KB, BL], BF16, tag="h0", name=f"h0_{s}")
        # act #2: Gelu (table load; net has slack vs the collective path)
        nc.scalar.activation(h_sb.rearrange("p k b -> p (k b)"),
                             h_ps.rearrange("p k b -> p (k b)"), GELU)

        # ---- softmax tail on tensor (w^T, scaled by -dt*beta_s) ----
        wT_ps = ps_small.tile([M, BL], F32, tag="sm", name=f"wT{s}")
        nc.tensor.transpose(wT_ps, w_n, ident[0:BL, 0:BL])
        wTs_sb = sb3.tile([M, BL], F32, tag="wTs", name=f"wTs{s}")
        nc.vector.tensor_scalar(wTs_sb, wT_ps, dtb8_sb[0:M, s:s + 1], None, ALU.mult)

        # ---- hidden layers, fully transposed flow: h^T -> h^T ----
        # hT[co,b] = gelu(sum_ci W[ci,co]^T hT[ci,b] + hb[co]); no casts,
        # no transposes between layers.
        for l in range(NH):
            hT_ps = ps_net.tile([128, KB, BL], F32, tag="h_ps", bufs=1,
                                name=f"hT{s}_{l}")
            for ko in range(KB):
                for ki in range(KB):
                    nc.tensor.matmul(hT_ps[:, ko, :],
                                     lhsT=hW_sb[:, l, ki, ko, :],
                                     rhs=h_sb[:, ki, :],
                                     start=(ki == 0), stop=False)
                nc.tensor.matmul(hT_ps[:, ko, :],
                                 lhsT=hb_sb[0:1, l * C + 128 * ko:
                                            l * C + 128 * (ko + 1)],
                                 rhs=ones_row_bf[0:1, 0:BL],
                                 start=False, stop=True)
            hn_sb = sb2.tile([128, KB, BL], BF16, tag=f"h{l + 1}",
                             name=f"hn_sb{s}_{l}")
            nc.scalar.activation(hn_sb.rearrange("p k b -> p (k b)"),
                                 hT_ps.rearrange("p k b -> p (k b)"), GELU)
            h_sb = hn_sb

        # ---- pairwise d2, local columns: psum = -2*(d2+A) ----
        d2l_ps = ps_d2l.tile([128, KB, BL], F32, tag="d2l", name=f"d2l{s}")
        for k in range(KB):
            nc.tensor.matmul(d2l_ps[:, k, :], lhsT=xall[:, 2 * k:2 * k + 2, :],
                             rhs=xT_locN2, start=True, stop=False)
            nc.tensor.matmul(d2l_ps[:, k, :], lhsT=ones_row_bf[0:1, 0:128],
                             rhs=x2locn2, start=False, stop=False)
            nc.tensor.matmul(d2l_ps[:, k, :],
                             lhsT=x2rowN2[0:1, 128 * k:128 * (k + 1)],
                             rhs=ones_row_bf[0:1, 0:BL], start=False, stop=True)

        # ---- repulsion kernel: kt = exp(-d2/h) ----
        kt_sb = sb2.tile([128, KB, BL], BF16, tag="kt", name=f"kt{s}")
        # act: Exp (table load #2; hides under the collective/d2l window)
        nc.scalar.activation(kt_sb.rearrange("p k b -> p (k b)"),
                             d2l_ps.rearrange("p k b -> p (k b)"), AF.Exp,
                             bias=bc[:, 1:2], scale=bc[:, 0:1])

        # ---- x rows scaled by c_h: xfe = c_h * x  (from -2x^T blocks) ----
        xft_ps = ps_net.tile([128, KB, BL], BF16, tag="h_ps", bufs=1,
                             name=f"xft{s}")
        for k in range(KB):
            nc.tensor.transpose(xft_ps[:, k, :], xall[:, 2 * k:2 * k + 2, :],
                                ident_bf[0:D, 0:D])
        xfe = sb2.tile([128, KB, BL], BF16, tag="xfe", name=f"xfe{s}")
        nc.vector.tensor_scalar(xfe.rearrange("p k b -> p (k b)"),
                                xft_ps.rearrange("p k b -> p (k b)"),
                                bc[:, 2:3], None, ALU.mult)

        # ---- U = dt*score + dt*out_b - dt*beta*(w@means) ----
        u_ps = ps_u.tile([BL, D], F32, tag="u", name=f"u{s}")
        for ki in range(KB):
            nc.tensor.matmul(u_ps, lhsT=h_sb[:, ki, :], rhs=outWs_sb[:, ki, :],
                             start=(ki == 0), stop=False)
        nc.tensor.matmul(u_ps, lhsT=ones_row_bf[0:1, 0:BL], rhs=outbs_row,
                         start=False, stop=False)
        nc.tensor.matmul(u_ps, lhsT=wTs_sb, rhs=means_sb, start=False, stop=False)

        # ---- kxr = c_h * K@x ; rch = c_h * r ----
        for k in range(KB):
            nc.tensor.matmul(u_ps, lhsT=kt_sb[:, k, :], rhs=xfe[:, k, :],
                             start=False, stop=(k == KB - 1))
        chcol_bf = sb3.tile([128, 1], BF16, tag="chcol", name=f"chcol{s}")
        nc.vector.tensor_copy(chcol_bf, bc[:, 3:4])
        rch_ps = ps_small.tile([BL, 1], F32, tag="sm", name=f"rch{s}")
        for k in range(KB):
            nc.tensor.matmul(rch_ps, lhsT=kt_sb[:, k, :], rhs=chcol_bf,
                             start=(k == 0), stop=(k == KB - 1))

        # ---- update: new = x*(1-dt+c_h*r) + noise - U - c_h*K@x ----
        alpha = sb3.tile([BL, 1], F32, tag="alpha", name=f"alpha{s}")
        nc.vector.tensor_tensor(alpha, rch_ps, omd_col[0:BL, 0:1], ALU.add)
        t1 = sb3.tile([BL, D], F32, tag="t1", name=f"t1_{s}")
        nc.vector.tensor_scalar(t1, x_loc, alpha, None, ALU.mult)
        t2 = sb3.tile([BL, D], F32, tag="t2", name=f"t2_{s}")
        nc.vector.tensor_tensor(t2, t1, noise_sb[:, s, :], ALU.add)
        new_x = sb2.tile([BL, D], F32, tag="x_loc", name=f"x{s + 1}")
        nc.vector.tensor_tensor(new_x, t2, u_ps, ALU.subtract)
        nc.scalar.dma_start(out=traj_d[s], in_=new_x)

        # ---- stage + post AllGather for step s+1 (one DMA: [-2x^T | x2]) ----
        if s + 1 < NB:
            nxT_ps = ps_small.tile([D, BL], F32, tag="sm", name=f"nxT{s + 1}")
            nc.tensor.transpose(nxT_ps, new_x, ident[0:BL, 0:BL])
            stg = sb2.tile([D, BL], BF16, tag="stg", name=f"stg{s + 1}")
            nc.vector.tensor_scalar(stg, nxT_ps, -2.0, None, ALU.mult)
            nc.sync.dma_start(
                out=agin[s + 1].ap()[0:AGW].rearrange("(p b) -> p b", p=D),
                in_=stg)
            nc.gpsimd.collective_compute(
                "AllGather", ALU.bypass, replica_groups=[list(range(NCORES))],
                ins=[agin[s + 1].ap().opt()], outs=[agout[s + 1].ap().opt()])
            nxT_loc = sb2.tile([D, BL], F32, tag="xT_loc", name=f"xT{s + 1}")
            nc.vector.tensor_copy(nxT_loc, nxT_ps)
            sqnT = scratch.tile([D, BL], F32, tag="sqnT", name=f"sqnT{s + 1}")
            nc.vector.tensor_tensor(sqnT, nxT_ps, nxT_loc, ALU.mult)
            x2l_ps = ps_small.tile([1, BL], F32, tag="sm", name=f"x2l{s + 1}")
            nc.tensor.matmul(x2l_ps, lhsT=ones_col[0:D, 0:1], rhs=sqnT,
                             start=True, stop=True)
            nx2locn2 = sb2.tile([1, BL], BF16, tag="x2locn2", name=f"x2n2_{s + 1}")
            nc.vector.tensor_scalar(nx2locn2, x2l_ps, -2.0, -2.0,
                                    ALU.mult, ALU.add)
            x_loc, xT_loc = new_x, nxT_loc
            xT_locN2 = stg
            x2locn2 = nx2locn2

        # ---- 2-step-stale bandwidth: process the PREVIOUS step's gather
        # (bc for step s+1 = h(x_{s-1})); issued after the stage/trigger so
        # its tensor/act work lands in this step's idle windows.
        if 1 <= s <= NB - 2:
            d2f_ps = ps_d2f.tile([128, B], F32, tag="d2f", name=f"d2f{s}")
            nc.tensor.matmul(d2f_ps, lhsT=xall_prev[:, 0:2, :],
                             rhs=xall_prev.rearrange("d c b -> d (c b)"),
                             start=True, stop=False)
            nc.tensor.matmul(d2f_ps, lhsT=ones_row_bf[0:1, 0:128],
                             rhs=x2row_prev,
                             start=False, stop=True)
            x2c_ps = ps_small.tile([128, 1], F32, tag="sm", name=f"x2c{s}")
            nc.tensor.matmul(x2c_ps, lhsT=x2row_prev[0:1, 0:128],
                             rhs=ones_col_bf[0:1, 0:1], start=True, stop=True)
            x2colP = sb3.tile([128, 1], F32, tag="x2colP", name=f"x2colP{s}")
            nc.vector.tensor_scalar(x2colP, x2c_ps, -0.5, None, ALU.mult)
            dsums = sb3.tile([128, 2], F32, tag="dsums", name=f"dsums{s}")
            zscr = scratch.tile([128, B], BF16, tag="zscr", name=f"zscr{s}")
            zscr2 = scratch.tile([128, B], BF16, tag="zscr2", name=f"zscr2{s}")
            # act (exp table, Identity+Square are in every table): z, z^2 sums
            nc.scalar.activation(zscr, d2f_ps, AF.Identity, bias=x2colP,
                                 scale=-0.5, accum_out=dsums[:, 0:1])
            nc.scalar.activation(zscr2, d2f_ps, AF.Square, bias=x2colP,
                                 scale=-0.5, accum_out=dsums[:, 1:2])
            sum12_ps = ps_small.tile([1, 2], F32, tag="sm", name=f"sum12{s}")
            nc.tensor.matmul(sum12_ps, lhsT=ones_col, rhs=dsums, start=True,
                             stop=True)
            # scalar chain: m=(S0/RS), m2=(S1/RS); q=(m2/m^2-1)/8
            # P = m*(1-q)^2 ~ E[sqrt(z)]^2 ; hL = P - A + A^2/(4P); bc=row4/hL
            mrow = sb3.tile([1, 2], F32, tag="mrow", name=f"mrow{s}")
            nc.vector.tensor_scalar(mrow, sum12_ps, 1.0 / float(RSUB), None,
                                    ALU.mult)
            msq = sb3.tile([1, 1], F32, tag="msq", name=f"msq{s}")
            nc.gpsimd.tensor_tensor(msq, mrow[0:1, 0:1], mrow[0:1, 0:1], ALU.mult)
            rmsq = sb3.tile([1, 1], F32, tag="rmsq", name=f"rmsq{s}")
            nc.vector.reciprocal(rmsq, msq)
            t2m = sb3.tile([1, 1], F32, tag="t2m", name=f"t2m{s}")
            nc.gpsimd.tensor_tensor(t2m, mrow[0:1, 1:2], rmsq, ALU.mult)
            uq = sb3.tile([1, 1], F32, tag="uq", name=f"uq{s}")
            nc.gpsimd.tensor_scalar(uq, t2m, -0.125, 1.125, ALU.mult, ALU.add)
            uq2 = sb3.tile([1, 1], F32, tag="uq2", name=f"uq2{s}")
            nc.gpsimd.tensor_tensor(uq2, uq, uq, ALU.mult)
            Pm = sb3.tile([1, 1], F32, tag="Pm", name=f"Pm{s}")
            nc.gpsimd.tensor_tensor(Pm, uq2, mrow[0:1, 0:1], ALU.mult)
            rP = sb3.tile([1, 1], F32, tag="rP", name=f"rP{s}")
            nc.vector.reciprocal(rP, Pm)
            z1 = sb3.tile([1, 1], F32, tag="z1", name=f"z1{s}")
            nc.gpsimd.tensor_scalar(z1, rP, EPS_A * EPS_A / 4.0, -EPS_A,
                                    ALU.mult, ALU.add)
            hL = sb3.tile([1, 1], F32, tag="hL", name=f"hL{s}")
            nc.gpsimd.tensor_tensor(hL, z1, Pm, ALU.add)
            rhL = sb3.tile([1, 1], F32, tag="rhL", name=f"rhL{s}")
            nc.vector.reciprocal(rhL, hL)
            rep4 = sb3.tile([1, 4], F32, tag="rep4", name=f"rep4{s}")
            nc.gpsimd.tensor_scalar(rep4, row4_sb, rhL, None, ALU.mult)
            nbc_ps = ps_small.tile([128, 4], F32, tag="sm", name=f"nbc{s}")
            nc.tensor.matmul(nbc_ps, lhsT=ones_row, rhs=rep4, start=True,
                             stop=True)
            nbc = sb2.tile([128, 4], F32, tag="bc", name=f"bc{s + 2}")
            nc.vector.tensor_copy(nbc, nbc_ps)
            bc_next = nbc
        # (s == 0 keeps bc_next = bc0 for step 1; s == NB-1 ends the loop)
        xall_prev, x2row_prev = xall, x2rowN2


# ======================================================================
# Host-side wrapper: prep + shard inputs, run SPMD on 8 cores, gather.
# ======================================================================
_CACHE = {}


def _get_nc():
    if "nc" not in _CACHE:
        _CACHE["nc"] = build_nc()
    return _CACHE["nc"]


def _np_gelu(x):
    return 0.5 * x * (1.0 + np.tanh(np.sqrt(2.0 / np.pi)
                                    * (x + 0.044715 * x ** 3)))


def _prep(inputs):
    """Host-side input-only transforms shared by all cores."""
    import ml_dtypes
    bf16 = ml_dtypes.bfloat16
    f32 = np.float32
    g = {}
    dt = float(np.asarray(inputs["eps"], np.float64)[0])
    x0 = np.asarray(inputs["particles"], np.float64)          # [B, D]

    # betas
    sig = 1.0 / (1.0 + np.exp(-np.asarray(inputs["grid_t"], np.float64)))
    betas = np.concatenate([[0.0], np.cumsum(sig)]) / sig.sum()

    # time-embedding table: te_s + in_b  [NB, C]
    coeff = np.linspace(0.1, 100.0, C, dtype=np.float64)[None, :]
    phase = np.asarray(inputs["phase"], np.float64)
    tW1 = np.asarray(inputs["t_W1"], np.float64)
    tW2 = np.asarray(inputs["t_W2"], np.float64)
    TE = np.zeros((NB, C))
    for s in range(NB):
        emb = coeff * s + phase
        temb = np.concatenate([np.sin(emb), np.cos(emb)], -1)
        te = _np_gelu(temb @ tW1 + np.asarray(inputs["t_b1"], np.float64)) \
            @ tW2 + np.asarray(inputs["t_b2"], np.float64)
        TE[s] = te + np.asarray(inputs["in_b"], np.float64)
    g["te_bf"] = TE.astype(f32).astype(bf16).reshape(1, NB * C)

    g["inWs_bf"] = (-0.5 * np.asarray(inputs["in_W"], f32)).astype(bf16)
    hW = np.asarray(inputs["h_W"], f32)                        # [NH, C, C]
    g["hW_bf"] = np.ascontiguousarray(
        hW.reshape(NH, KB, 128, KB, 128).transpose(2, 0, 1, 3, 4)
        .reshape(128, -1)).astype(bf16)
    g["hb_bf"] = np.asarray(inputs["h_b"], f32).astype(bf16).reshape(1, NH * C)
    outW = np.asarray(inputs["out_W"], f32)                    # [C, D]
    g["outWs_bf"] = np.ascontiguousarray(
        (dt * outW).reshape(KB, 128, D).transpose(1, 0, 2).reshape(128, -1)
    ).astype(bf16)
    g["outbs_bf"] = (dt * np.asarray(inputs["out_b"], f32)[None, :]).astype(bf16)

    means = np.asarray(inputs["target_means"], f32)
    g["means"] = means
    g["meansT"] = np.ascontiguousarray(means.T)
    g["negmu2"] = (-0.5 * (means.astype(np.float64) ** 2).sum(-1)[None, :]
                   ).astype(f32)
    g["dtb8"] = np.tile((-dt * betas[:NB]).astype(f32)[None, :], (M, 1))
    row4 = np.array([[0.5 * LOGN, EPS_A * LOGN, -0.05 * dt * LOGN,
                      0.1 * dt * LOGN]], np.float64)
    g["row4"] = row4.astype(f32)

    # host bandwidth for steps 0 and 1: hL = h*logn from x0 subsample
    f = x0[:128, None, :] - x0[None, :, :]
    z = (f * f).sum(-1) + EPS_A
    m, m2 = z.mean(), (z * z).mean()
    q = (m2 - m * m) / (8.0 * m * m)
    P = m * (1.0 - q) ** 2
    hL0 = P - EPS_A + EPS_A * EPS_A / (4.0 * P)
    g["bcast0_row"] = (row4 / hL0).astype(f32)
    g["omd_col"] = np.full((128, 1), 1.0 - dt, f32)

    # full-particle tiles for step 0
    x0f = x0.astype(f32)
    g["xall0"] = np.ascontiguousarray(-2.0 * x0f.T).astype(bf16)
    x2 = (x0f * x0f).sum(-1) + 1.0
    g["x2rowN2_0"] = (-2.0 * x2[None, :]).astype(bf16)

    # noise, pre-scaled, [B, NB, D]
    noi = np.asarray(inputs["noises"], f32) * np.float32(np.sqrt(2.0 * dt))
    g["noises_all"] = np.ascontiguousarray(noi.transpose(1, 0, 2))
    g["x0f"] = x0f
    return g


def _shard(g, c):
    import ml_dtypes
    bf16 = ml_dtypes.bfloat16
    sl = slice(c * BL, (c + 1) * BL)
    m = {k: g[k] for k in ["inWs_bf", "te_bf", "hW_bf", "hb_bf", "outWs_bf",
                           "outbs_bf", "means", "meansT", "negmu2", "dtb8",
                           "row4", "bcast0_row", "omd_col", "xall0",
                           "x2rowN2_0"]}
    x0l = np.ascontiguousarray(g["x0f"][sl])
    m["x0_loc"] = x0l
    m["xT0_loc"] = np.ascontiguousarray(x0l.T)
    m["xT0_locN2"] = np.ascontiguousarray(-2.0 * x0l.T).astype(bf16)
    x2 = (x0l * x0l).sum(-1) + 1.0
    m["x2locn2_0"] = (-2.0 * x2[None, :]).astype(bf16)
    m["noises_s"] = np.ascontiguousarray(g["noises_all"][sl])
    return m


def run(inputs, trace=False, trace_cores=None):
    from concourse.bass_utils import run_bass_kernel_spmd
    nc = _get_nc()
    g = _prep(inputs)
    in_maps = [_shard(g, c) for c in range(NCORES)]
    res = run_bass_kernel_spmd(nc, in_maps, core_ids=list(range(NCORES)),
                               trace=trace, trace_cores=trace_cores)
    out = np.zeros((NB + 1, B, D), np.float32)
    out[0] = np.asarray(inputs["particles"], np.float32)
    for c in range(NCORES):
        out[1:, c * BL:(c + 1) * BL, :] = \
            np.asarray(res.results[c]["traj"]).reshape(NB, BL, D)
    return out, res


def kernel(**inputs):
    return run(inputs)[0]


# revision 24
# speedup vs baseline: 1.2479x; 1.0035x over previous
"""Bass/Tile kernel for nn_CMCD (annealed Langevin sampler with SVGD repulsion).

SPMD over 8 cores, data-parallel over the particle batch (64 rows/core).

Structure (v2):
- Host precomputes all input-only transforms: time-embedding table, betas,
  weight layouts/casts, noise prescale, and the step-0 particle tiles
  (so step 0 needs no collective).
- A tiny warm-up AllGather fires at t~0 to absorb collective mesh-init /
  core-start skew while weights stream in.
- Per step s>=1: AllGather of x_s posted at the end of step s-1; the
  score net + mixture-gradient run in its shadow; repulsion from the
  gathered particles; fused update.
- Activation-table discipline: steady-state act functions are only
  {Exp, Gelu, Identity, Square} ordered as [exp-block][gelu-block] per
  step -> 2 table loads/step.
- Bandwidth (SVGD median heuristic) replaced by a calibrated sqrt-free
  estimator computed from mean/var of d2 over a 128x512 subsample, one
  step stale (h_s = h(x_{s-1})); step 0/1 bandwidth comes from the host.
  Validated end-to-end at rel err ~2e-5 vs the jax reference.
"""
import numpy as np
from contextlib import ExitStack

import concourse.bass as bass
import concourse.bacc as bacc
import concourse.tile as tile
from concourse import mybir
from concourse.masks import make_identity

D, C, NB, NH, M = 64, 512, 8, 3, 8
B = 512
NCORES = 8
BL = B // NCORES  # 64
KB = C // 128     # 4 channel blocks
LOGN = float(np.log(B))
RSUB = 128 * B    # subsample count for the bandwidth (rows 0..127)
AGW = BL * D  # flat AllGather payload words per core (x2 recomputed receiver-side)
NJUNK = 400
EPS_A = 2.0        # total d2 shift (bf16-safety); corrected exactly
F32 = mybir.dt.float32
BF16 = mybir.dt.bfloat16
AF = mybir.ActivationFunctionType
ALU = mybir.AluOpType
GELU = AF.Gelu_apprx_tanh


def build_nc(compile=True):
    nc = bacc.Bacc("TRN2", target_bir_lowering=False, debug=False,
                   num_devices=NCORES)

    t = {}
    def din(name, shape, dtype):
        t[name] = nc.dram_tensor(name, shape, dtype, kind="ExternalInput")

    # ---- per-core state inputs ----
    din("x0_loc", [BL, D], F32)
    din("xT0_loc", [D, BL], F32)
    din("xT0_locN2", [D, BL], BF16)
    din("x2locn2_0", [1, BL], BF16)
    din("xall0", [D, B], BF16)        # -2 * x0^T, all particles
    din("x2rowN2_0", [1, B], BF16)    # -2*(|x0_j|^2 + 1), all particles
    din("noises_s", [BL, NB, D], F32)  # pre-scaled by sqrt(2 dt)
    # ---- weights / tables (host-prepped) ----
    din("inWs_bf", [D, C], BF16)       # -0.5 * in_W
    din("te_bf", [1, NB * C], BF16)    # te_s + in_b, flat row
    din("hW_bf", [128, NH * KB * C], BF16)
    din("hb_bf", [1, NH * C], BF16)
    din("outWs_bf", [128, KB * D], BF16)  # dt * out_W
    din("outbs_bf", [1, D], BF16)         # dt * out_b
    din("cpack", [128, 97], F32)  # packed small f32 constants (see _prep)

    traj_d = nc.dram_tensor("traj", [NB, BL, D], F32, kind="ExternalOutput")
    t["traj_d"] = traj_d

    # collective bounce buffers: steps 1..NB-1, plus a warm-up dummy
    t["agin"] = [None] + [nc.dram_tensor(f"agin{s}", [AGW], BF16)
                          for s in range(1, NB)]
    t["agout"] = [None] + [nc.dram_tensor(f"agout{s}", [NCORES, AGW], BF16,
                                          addr_space="Shared")
                           for s in range(1, NB)]
    t["dd_in"] = nc.dram_tensor("dd_in", [64], BF16)
    t["dd_out"] = nc.dram_tensor("dd_out", [NCORES, 64], BF16,
                                 addr_space="Shared")

    with tile.TileContext(nc) as tc, ExitStack() as ctx:
        _body(ctx, tc, nc, t)
    if compile:
        nc.compile()
    return nc


def _body(ctx, tc, nc, t):
    traj_d, agin, agout = t["traj_d"], t["agin"], t["agout"]

    const = ctx.enter_context(tc.tile_pool(name="const", bufs=1))
    wpool = ctx.enter_context(tc.tile_pool(name="wpool", bufs=1))
    sb2 = ctx.enter_context(tc.tile_pool(name="sb2", bufs=2))
    sb3 = ctx.enter_context(tc.tile_pool(name="sb3", bufs=3))
    scratch = ctx.enter_context(tc.tile_pool(name="scratch", bufs=2))
    ps_small = ctx.enter_context(tc.tile_pool(name="ps_small", bufs=2, space="PSUM"))
    ps_d2f = ctx.enter_context(tc.tile_pool(name="ps_d2f", bufs=1, space="PSUM"))
    ps_d2l = ctx.enter_context(tc.tile_pool(name="ps_d2l", bufs=1, space="PSUM"))
    ps_u = ctx.enter_context(tc.tile_pool(name="ps_u", bufs=1, space="PSUM"))
    ps_cmp = ctx.enter_context(tc.tile_pool(name="ps_cmp", bufs=1, space="PSUM"))
    ps_net = ctx.enter_context(tc.tile_pool(name="ps_net", bufs=2, space="PSUM"))

    # ---------------- warm-up collective: very first instruction ----------------
    nc.gpsimd.collective_compute(
        "AllGather", ALU.bypass, replica_groups=[list(range(NCORES))],
        ins=[t["dd_in"].ap().opt()], outs=[t["dd_out"].ap().opt()])

    # ---------------- input DMAs (3 queues, ordered by first use) ----------------
    # queue A (sync): step-0 particle tiles + noises
    x0_loc = wpool.tile([BL, D], F32)
    nc.sync.dma_start(out=x0_loc, in_=t["x0_loc"][:, :])
    xT0_loc = wpool.tile([D, BL], F32)
    nc.sync.dma_start(out=xT0_loc, in_=t["xT0_loc"][:, :])
    xT0_locN2 = wpool.tile([D, BL], BF16)
    nc.sync.dma_start(out=xT0_locN2, in_=t["xT0_locN2"][:, :])
    x2locn2_0 = wpool.tile([1, BL], BF16)
    nc.sync.dma_start(out=x2locn2_0, in_=t["x2locn2_0"][:, :])
    xall0 = wpool.tile([D, NCORES, BL], BF16)
    nc.sync.dma_start(out=xall0, in_=t["xall0"].ap().rearrange(
        "d (c b) -> d c b", c=NCORES))
    x2rowN2_0 = wpool.tile([1, B], BF16)
    nc.sync.dma_start(out=x2rowN2_0, in_=t["x2rowN2_0"][:, :])
    # queue A continues: second half of hW (noises follow - needed later)
    hW_sb = wpool.tile([128, NH, KB, KB, 128], BF16)
    # queue B (scalar): small weights in use order
    inWs_bf = wpool.tile([D, C], BF16)
    nc.scalar.dma_start(out=inWs_bf, in_=t["inWs_bf"][:, :])
    te_bf = wpool.tile([1, NB * C], BF16)
    nc.scalar.dma_start(out=te_bf[0:1, 0:NB * C // 2],
                        in_=t["te_bf"][0:1, 0:NB * C // 2])
    nc.sync.dma_start(out=te_bf[0:1, NB * C // 2:],
                      in_=t["te_bf"][0:1, NB * C // 2:])
    cpack_sb = wpool.tile([128, 97], F32)
    nc.scalar.dma_start(out=cpack_sb, in_=t["cpack"][:, :])
    meansT_sb = cpack_sb[0:D, 0:M]
    means_sb = cpack_sb[0:M, 8:8 + D]
    negmu2_row = cpack_sb[0:1, 72:72 + M]
    dtb8_sb = cpack_sb[0:M, 80:80 + NB]
    row4_sb = cpack_sb[0:1, 88:92]
    bcast0_row = cpack_sb[0:1, 92:96]
    omd_col = cpack_sb[:, 96:97]
    # hW split across queues A and B (~750KB each), ahead of late-use weights
    hWr = t["hW_bf"].ap().rearrange("p (l a b q) -> p l a b q", l=NH, a=KB,
                                    b=KB)
    nc.scalar.dma_start(out=hW_sb[:, 0:2, :, :, :], in_=hWr[:, 0:2, :, :, :])
    nc.sync.dma_start(out=hW_sb[:, 2:NH, :, :, :], in_=hWr[:, 2:NH, :, :, :])
    hb_sb = wpool.tile([1, NH * C], BF16)
    nc.scalar.dma_start(out=hb_sb, in_=t["hb_bf"][:, :])
    outWs_sb = wpool.tile([128, KB, D], BF16)
    nc.scalar.dma_start(out=outWs_sb, in_=t["outWs_bf"].ap().rearrange(
        "p (k d) -> p k d", k=KB))
    outbs_row = wpool.tile([1, D], BF16)
    nc.scalar.dma_start(out=outbs_row, in_=t["outbs_bf"][:, :])
    noise_sb = wpool.tile([BL, NB, D], F32)
    nc.sync.dma_start(out=noise_sb, in_=t["noises_s"][:, :, :])

    # ---------------- constants ----------------
    ident = const.tile([128, 128], F32)
    make_identity(nc, ident)
    ident_bf = const.tile([128, 128], BF16)
    nc.vector.tensor_copy(ident_bf, ident)
    ones_col = const.tile([128, 1], F32)
    nc.vector.memset(ones_col, 1.0)
    ones_row = const.tile([1, 128], F32)
    nc.vector.memset(ones_row, 1.0)
    ones_row_bf = const.tile([1, 128], BF16)
    nc.vector.memset(ones_row_bf, 1.0)
    ones_col_bf = const.tile([128, 1], BF16)
    nc.vector.memset(ones_col_bf, 1.0)

    # broadcast bcast0_row -> [128, 4] (used by steps 0 and 1)
    bc0_ps = ps_small.tile([128, 4], F32, tag="sm", name="bc0_ps")
    nc.tensor.matmul(bc0_ps, lhsT=ones_row, rhs=bcast0_row, start=True, stop=True)
    bc0 = const.tile([128, 4], F32)
    nc.vector.tensor_copy(bc0, bc0_ps)

    # ---------------- per-step state handles ----------------
    x_loc = x0_loc
    xT_loc = xT0_loc
    xT_locN2 = xT0_locN2
    x2locn2 = x2locn2_0
    bc_next = bc0  # bandwidth broadcast for the *next* issued step

    for s in range(NB):
        bc = bc_next
        # ---- gathered particle tiles ----
        if s == 0:
            xall = xall0
            x2rowN2 = x2rowN2_0
        else:
            xall = sb2.tile([D, NCORES, BL], BF16, tag="xall", name=f"xall{s}")
            for half, eng in ((0, nc.sync), (1, nc.scalar)):
                eng.dma_start(
                    out=xall[:, half * 4:(half + 1) * 4, :],
                    in_=bass.AP(tensor=agout[s].ap().tensor,
                                offset=half * 4 * AGW,
                                ap=[[BL, D], [AGW, 4], [1, BL]]))
            # recompute -2*(|x_j|^2+1) from the gathered -2x^T (gpsimd+PE,
            # off the stage/trigger critical path)
            sq = scratch.tile([D, B], F32, tag="sq", name=f"sq{s}")
            nc.gpsimd.tensor_tensor(sq, xall.rearrange("d c b -> d (c b)"),
                                    xall.rearrange("d c b -> d (c b)"),
                                    ALU.mult)
            x2r_ps = ps_small.tile([1, B], F32, tag="sm", name=f"x2r_ps{s}")
            nc.tensor.matmul(x2r_ps, lhsT=ones_col[0:D, 0:1], rhs=sq,
                             start=True, stop=True)
            x2rowN2 = sb3.tile([1, B], BF16, tag="x2rowN2", name=f"x2r{s}")
            nc.vector.tensor_scalar(x2rowN2, x2r_ps, -0.5, -2.0,
                                    ALU.mult, ALU.add)

        # ================= TENSOR (+ matching act/vector) =================
        # ---- mixture-component logits (feeds softmax exp, first act op) ----
        comp_ps = ps_cmp.tile([BL, M], F32, tag="cmp", name=f"comp{s}")
        nc.tensor.matmul(comp_ps, lhsT=xT_loc, rhs=meansT_sb, start=True, stop=False)
        nc.tensor.matmul(comp_ps, lhsT=ones_row[0:1, 0:BL], rhs=negmu2_row,
                         start=False, stop=True)
        negmax = sb3.tile([BL, 1], F32, tag="negmax", name=f"negmax{s}")
        nc.vector.tensor_reduce(negmax, comp_ps, axis=mybir.AxisListType.X,
                                op=ALU.max, negate=True)
        # softmax exps via Tanh (resident in the gelu table):
        # e^-w = (1-tanh(w/2))/(1+tanh(w/2)), w = -(comp+negmax) >= 0
        halfmax = sb3.tile([BL, 1], F32, tag="halfmax", name=f"halfmax{s}")
        nc.vector.tensor_scalar(halfmax, negmax, -0.5, None, ALU.mult)
        tsm = sb3.tile([BL, M], F32, tag="tsm", name=f"tsm{s}")
        nc.scalar.activation(tsm, comp_ps, AF.Tanh, bias=halfmax, scale=-0.5)
        numt = sb3.tile([BL, M], F32, tag="numt", name=f"numt{s}")
        nc.vector.tensor_scalar(numt, tsm, -1.0, 1.0, ALU.mult, ALU.add)
        dent = sb3.tile([BL, M], F32, tag="dent", name=f"dent{s}")
        nc.vector.tensor_scalar(dent, tsm, 1.0, 1.0, ALU.mult, ALU.add)
        rden = sb3.tile([BL, M], F32, tag="rden", name=f"rden{s}")
        nc.vector.reciprocal(rden, dent)
        w_un = sb3.tile([BL, M], F32, tag="w_un", name=f"w_un{s}")
        nc.vector.tensor_tensor(w_un, numt, rden, ALU.mult)
        sumexp = sb3.tile([BL, 1], F32, tag="sumexp", name=f"sumexp{s}")
        nc.vector.tensor_reduce(sumexp, w_un, axis=mybir.AxisListType.X,
                                op=ALU.add)
        rcp = sb3.tile([BL, 1], F32, tag="rcp", name=f"rcp{s}")
        nc.vector.reciprocal(rcp, sumexp)
        w_n = sb3.tile([BL, M], F32, tag="w_n", name=f"w_n{s}")
        nc.vector.tensor_scalar(w_n, w_un, rcp, None, ALU.mult)

        # ---- score net layer 1: h^T = (x @ in_W)^T + te (gelu) ----
        h_ps = ps_net.tile([128, KB, BL], F32, tag="h_ps", bufs=1, name=f"h_ps{s}")
        for ko in range(KB):
            nc.tensor.matmul(h_ps[:, ko, :],
                             lhsT=inWs_bf[:, 128 * ko:128 * (ko + 1)],
                             rhs=xT_locN2, start=True, stop=False)
            nc.tensor.matmul(h_ps[:, ko, :],
                             lhsT=te_bf[0:1, s * C + 128 * ko:
                                        s * C + 128 * (ko + 1)],
                             rhs=ones_row_bf[0:1, 0:BL], start=False, stop=True)
        h_sb = sb2.tile([128, KB, BL], BF16, tag="h0", name=f"h0_{s}")
        # act #2: Gelu (table load; net has slack vs the collective path)
        nc.scalar.activation(h_sb.rearrange("p k b -> p (k b)"),
                             h_ps.rearrange("p k b -> p (k b)"), GELU)

        # ---- softmax tail on tensor (w^T, scaled by -dt*beta_s) ----
        wT_ps = ps_small.tile([M, BL], F32, tag="sm", name=f"wT{s}")
        nc.tensor.transpose(wT_ps, w_n, ident[0:BL, 0:BL])
        wTs_sb = sb3.tile([M, BL], F32, tag="wTs", name=f"wTs{s}")
        nc.vector.tensor_scalar(wTs_sb, wT_ps,
                                cpack_sb[0:M, 80 + s:81 + s], None, ALU.mult)

        # ---- hidden layers, fully transposed flow: h^T -> h^T ----
        # hT[co,b] = gelu(sum_ci W[ci,co]^T hT[ci,b] + hb[co]); no casts,
        # no transposes between layers.
        for l in range(NH):
            hT_ps = ps_net.tile([128, KB, BL], F32, tag="h_ps", bufs=1,
                                name=f"hT{s}_{l}")
            for ko in range(KB):
                for ki in range(KB):
                    nc.tensor.matmul(hT_ps[:, ko, :],
                                     lhsT=hW_sb[:, l, ki, ko, :],
                                     rhs=h_sb[:, ki, :],
                                     start=(ki == 0), stop=False)
                nc.tensor.matmul(hT_ps[:, ko, :],
                                 lhsT=hb_sb[0:1, l * C + 128 * ko:
                                            l * C + 128 * (ko + 1)],
                                 rhs=ones_row_bf[0:1, 0:BL],
                                 start=False, stop=True)
            hn_sb = sb2.tile([128, KB, BL], BF16, tag=f"h{l + 1}",
                             name=f"hn_sb{s}_{l}")
            nc.scalar.activation(hn_sb.rearrange("p k b -> p (k b)"),
                                 hT_ps.rearrange("p k b -> p (k b)"), GELU)
            h_sb = hn_sb

        # ---- pairwise d2, local columns: psum = -2*(d2+A) ----
        d2l_ps = ps_d2l.tile([128, KB, BL], F32, tag="d2l", name=f"d2l{s}")
        for k in range(KB):
            nc.tensor.matmul(d2l_ps[:, k, :], lhsT=xall[:, 2 * k:2 * k + 2, :],
                             rhs=xT_locN2, start=True, stop=False)
            nc.tensor.matmul(d2l_ps[:, k, :], lhsT=ones_row_bf[0:1, 0:128],
                             rhs=x2locn2, start=False, stop=False)
            nc.tensor.matmul(d2l_ps[:, k, :],
                             lhsT=x2rowN2[0:1, 128 * k:128 * (k + 1)],
                             rhs=ones_row_bf[0:1, 0:BL], start=False, stop=True)

        # ---- repulsion kernel: kt = exp(-d2/h) ----
        kt_sb = sb2.tile([128, KB, BL], BF16, tag="kt", name=f"kt{s}")
        # act: Exp (table load #2; hides under the collective/d2l window)
        nc.scalar.activation(kt_sb.rearrange("p k b -> p (k b)"),
                             d2l_ps.rearrange("p k b -> p (k b)"), AF.Exp,
                             bias=bc[:, 1:2], scale=bc[:, 0:1])

        # ---- x rows scaled by c_h: xfe = c_h * x  (from -2x^T blocks) ----
        xft_ps = ps_net.tile([128, KB, BL], BF16, tag="h_ps", bufs=1,
                             name=f"xft{s}")
        for k in range(KB):
            nc.tensor.transpose(xft_ps[:, k, :], xall[:, 2 * k:2 * k + 2, :],
                                ident_bf[0:D, 0:D])
        xfe = sb2.tile([128, KB, BL], BF16, tag="xfe", name=f"xfe{s}")
        nc.vector.tensor_scalar(xfe.rearrange("p k b -> p (k b)"),
                                xft_ps.rearrange("p k b -> p (k b)"),
                                bc[:, 2:3], None, ALU.mult)

        # ---- U = dt*score + dt*out_b - dt*beta*(w@means) ----
        u_ps = ps_u.tile([BL, D], F32, tag="u", name=f"u{s}")
        for ki in range(KB):
            nc.tensor.matmul(u_ps, lhsT=h_sb[:, ki, :], rhs=outWs_sb[:, ki, :],
                             start=(ki == 0), stop=False)
        nc.tensor.matmul(u_ps, lhsT=ones_row_bf[0:1, 0:BL], rhs=outbs_row,
                         start=False, stop=False)
        nc.tensor.matmul(u_ps, lhsT=wTs_sb, rhs=means_sb, start=False, stop=False)

        # ---- kxr = c_h * K@x ; rch = c_h * r ----
        for k in range(KB):
            nc.tensor.matmul(u_ps, lhsT=kt_sb[:, k, :], rhs=xfe[:, k, :],
                             start=False, stop=(k == KB - 1))
        chcol_bf = sb3.tile([128, 1], BF16, tag="chcol", name=f"chcol{s}")
        nc.vector.tensor_copy(chcol_bf, bc[:, 3:4])
        rch_ps = ps_small.tile([BL, 1], F32, tag="sm", name=f"rch{s}")
        for k in range(KB):
            nc.tensor.matmul(rch_ps, lhsT=kt_sb[:, k, :], rhs=chcol_bf,
                             start=(k == 0), stop=(k == KB - 1))

        # ---- update: new = x*(1-dt+c_h*r) + noise - U - c_h*K@x ----
        alpha = sb3.tile([BL, 1], F32, tag="alpha", name=f"alpha{s}")
        nc.vector.tensor_tensor(alpha, rch_ps, cpack_sb[0:BL, 96:97], ALU.add)
        t1 = sb3.tile([BL, D], F32, tag="t1", name=f"t1_{s}")
        nc.vector.tensor_scalar(t1, x_loc, alpha, None, ALU.mult)
        t2 = sb3.tile([BL, D], F32, tag="t2", name=f"t2_{s}")
        nc.vector.tensor_tensor(t2, t1, noise_sb[:, s, :], ALU.add)
        new_x = sb2.tile([BL, D], F32, tag="x_loc", name=f"x{s + 1}")
        nc.vector.tensor_tensor(new_x, t2, u_ps, ALU.subtract)
        nc.scalar.dma_start(out=traj_d[s], in_=new_x)

        # ---- stage + post AllGather for step s+1 (one DMA: [-2x^T | x2]) ----
        if s + 1 < NB:
            nxT_ps = ps_small.tile([D, BL], F32, tag="sm", name=f"nxT{s + 1}")
            nc.tensor.transpose(nxT_ps, new_x, ident[0:BL, 0:BL])
            stg = sb2.tile([D, BL], BF16, tag="stg", name=f"stg{s + 1}")
            nc.vector.tensor_scalar(stg, nxT_ps, -2.0, None, ALU.mult)
            nc.sync.dma_start(
                out=agin[s + 1].ap()[0:AGW].rearrange("(p b) -> p b", p=D),
                in_=stg)
            nc.gpsimd.collective_compute(
                "AllGather", ALU.bypass, replica_groups=[list(range(NCORES))],
                ins=[agin[s + 1].ap().opt()], outs=[agout[s + 1].ap().opt()])
            nxT_loc = sb2.tile([D, BL], F32, tag="xT_loc", name=f"xT{s + 1}")
            nc.vector.tensor_copy(nxT_loc, nxT_ps)
            sqnT = scratch.tile([D, BL], F32, tag="sqnT", name=f"sqnT{s + 1}")
            nc.vector.tensor_tensor(sqnT, nxT_ps, nxT_loc, ALU.mult)
            x2l_ps = ps_small.tile([1, BL], F32, tag="sm", name=f"x2l{s + 1}")
            nc.tensor.matmul(x2l_ps, lhsT=ones_col[0:D, 0:1], rhs=sqnT,
                             start=True, stop=True)
            nx2locn2 = sb2.tile([1, BL], BF16, tag="x2locn2", name=f"x2n2_{s + 1}")
            nc.vector.tensor_scalar(nx2locn2, x2l_ps, -2.0, -2.0,
                                    ALU.mult, ALU.add)
            x_loc, xT_loc = new_x, nxT_loc
            xT_locN2 = stg
            x2locn2 = nx2locn2

        # ---- 2-step-stale bandwidth: process the PREVIOUS step's gather
        # (bc for step s+1 = h(x_{s-1})); issued after the stage/trigger so
        # its tensor/act work lands in this step's idle windows.
        if 1 <= s <= NB - 2:
            d2f_ps = ps_d2f.tile([128, B], F32, tag="d2f", name=f"d2f{s}")
            nc.tensor.matmul(d2f_ps, lhsT=xall_prev[:, 0:2, :],
                             rhs=xall_prev.rearrange("d c b -> d (c b)"),
                             start=True, stop=False)
            nc.tensor.matmul(d2f_ps, lhsT=ones_row_bf[0:1, 0:128],
                             rhs=x2row_prev,
                             start=False, stop=True)
            x2c_ps = ps_small.tile([128, 1], F32, tag="sm", name=f"x2c{s}")
            nc.tensor.matmul(x2c_ps, lhsT=x2row_prev[0:1, 0:128],
                             rhs=ones_col_bf[0:1, 0:1], start=True, stop=True)
            x2colP = sb3.tile([128, 1], F32, tag="x2colP", name=f"x2colP{s}")
            nc.vector.tensor_scalar(x2colP, x2c_ps, -0.5, None, ALU.mult)
            dsums = sb3.tile([128, 2], F32, tag="dsums", name=f"dsums{s}")
            zscr = scratch.tile([128, B], BF16, tag="zscr", name=f"zscr{s}")
            zscr2 = scratch.tile([128, B], BF16, tag="zscr2", name=f"zscr2{s}")
            # act (exp table, Identity+Square are in every table): z, z^2 sums
            nc.scalar.activation(zscr, d2f_ps, AF.Identity, bias=x2colP,
                                 scale=-0.5, accum_out=dsums[:, 0:1])
            nc.scalar.activation(zscr2, d2f_ps, AF.Square, bias=x2colP,
                                 scale=-0.5, accum_out=dsums[:, 1:2])
            sum12_ps = ps_small.tile([1, 2], F32, tag="sm", name=f"sum12{s}")
            nc.tensor.matmul(sum12_ps, lhsT=ones_col, rhs=dsums, start=True,
                             stop=True)
            # scalar chain: m=(S0/RS), m2=(S1/RS); q=(m2/m^2-1)/8
            # P = m*(1-q)^2 ~ E[sqrt(z)]^2 ; hL = P - A + A^2/(4P); bc=row4/hL
            mrow = sb3.tile([1, 2], F32, tag="mrow", name=f"mrow{s}")
            nc.vector.tensor_scalar(mrow, sum12_ps, 1.0 / float(RSUB), None,
                                    ALU.mult)
            msq = sb3.tile([1, 1], F32, tag="msq", name=f"msq{s}")
            nc.gpsimd.tensor_tensor(msq, mrow[0:1, 0:1], mrow[0:1, 0:1], ALU.mult)
            rmsq = sb3.tile([1, 1], F32, tag="rmsq", name=f"rmsq{s}")
            nc.vector.reciprocal(rmsq, msq)
            t2m = sb3.tile([1, 1], F32, tag="t2m", name=f"t2m{s}")
            nc.gpsimd.tensor_tensor(t2m, mrow[0:1, 1:2], rmsq, ALU.mult)
            uq = sb3.tile([1, 1], F32, tag="uq", name=f"uq{s}")
            nc.gpsimd.tensor_scalar(uq, t2m, -0.125, 1.125, ALU.mult, ALU.add)
            uq2 = sb3.tile([1, 1], F32, tag="uq2", name=f"uq2{s}")
            nc.gpsimd.tensor_tensor(uq2, uq, uq, ALU.mult)
            Pm = sb3.tile([1, 1], F32, tag="Pm", name=f"Pm{s}")
            nc.gpsimd.tensor_tensor(Pm, uq2, mrow[0:1, 0:1], ALU.mult)
            rP = sb3.tile([1, 1], F32, tag="rP", name=f"rP{s}")
            nc.vector.reciprocal(rP, Pm)
            z1 = sb3.tile([1, 1], F32, tag="z1", name=f"z1{s}")
            nc.gpsimd.tensor_scalar(z1, rP, EPS_A * EPS_A / 4.0, -EPS_A,
                                    ALU.mult, ALU.add)
            hL = sb3.tile([1, 1], F32, tag="hL", name=f"hL{s}")
            nc.gpsimd.tensor_tensor(hL, z1, Pm, ALU.add)
            rhL = sb3.tile([1, 1], F32, tag="rhL", name=f"rhL{s}")
            nc.vector.reciprocal(rhL, hL)
            rep4 = sb3.tile([1, 4], F32, tag="rep4", name=f"rep4{s}")
            nc.gpsimd.tensor_scalar(rep4, row4_sb, rhL, None, ALU.mult)
            nbc_ps = ps_small.tile([128, 4], F32, tag="sm", name=f"nbc{s}")
            nc.tensor.matmul(nbc_ps, lhsT=ones_row, rhs=rep4, start=True,
                             stop=True)
            nbc = sb2.tile([128, 4], F32, tag="bc", name=f"bc{s + 2}")
            nc.vector.tensor_copy(nbc, nbc_ps)
            bc_next = nbc
        # (s == 0 keeps bc_next = bc0 for step 1; s == NB-1 ends the loop)
        xall_prev, x2row_prev = xall, x2rowN2


# ======================================================================
# Host-side wrapper: prep + shard inputs, run SPMD on 8 cores, gather.
# ======================================================================
_CACHE = {}


def _get_nc():
    if "nc" not in _CACHE:
        _CACHE["nc"] = build_nc()
    return _CACHE["nc"]


def _np_gelu(x):
    return 0.5 * x * (1.0 + np.tanh(np.sqrt(2.0 / np.pi)
                                    * (x + 0.044715 * x ** 3)))


def _prep(inputs):
    """Host-side input-only transforms shared by all cores."""
    import ml_dtypes
    bf16 = ml_dtypes.bfloat16
    f32 = np.float32
    g = {}
    dt = float(np.asarray(inputs["eps"], np.float64)[0])
    x0 = np.asarray(inputs["particles"], np.float64)          # [B, D]

    # betas
    sig = 1.0 / (1.0 + np.exp(-np.asarray(inputs["grid_t"], np.float64)))
    betas = np.concatenate([[0.0], np.cumsum(sig)]) / sig.sum()

    # time-embedding table: te_s + in_b  [NB, C]
    coeff = np.linspace(0.1, 100.0, C, dtype=np.float64)[None, :]
    phase = np.asarray(inputs["phase"], np.float64)
    tW1 = np.asarray(inputs["t_W1"], np.float64)
    tW2 = np.asarray(inputs["t_W2"], np.float64)
    TE = np.zeros((NB, C))
    for s in range(NB):
        emb = coeff * s + phase
        temb = np.concatenate([np.sin(emb), np.cos(emb)], -1)
        te = _np_gelu(temb @ tW1 + np.asarray(inputs["t_b1"], np.float64)) \
            @ tW2 + np.asarray(inputs["t_b2"], np.float64)
        TE[s] = te + np.asarray(inputs["in_b"], np.float64)
    g["te_bf"] = TE.astype(f32).astype(bf16).reshape(1, NB * C)

    g["inWs_bf"] = (-0.5 * np.asarray(inputs["in_W"], f32)).astype(bf16)
    hW = np.asarray(inputs["h_W"], f32)                        # [NH, C, C]
    g["hW_bf"] = np.ascontiguousarray(
        hW.reshape(NH, KB, 128, KB, 128).transpose(2, 0, 1, 3, 4)
        .reshape(128, -1)).astype(bf16)
    g["hb_bf"] = np.asarray(inputs["h_b"], f32).astype(bf16).reshape(1, NH * C)
    outW = np.asarray(inputs["out_W"], f32)                    # [C, D]
    g["outWs_bf"] = np.ascontiguousarray(
        (dt * outW).reshape(KB, 128, D).transpose(1, 0, 2).reshape(128, -1)
    ).astype(bf16)
    g["outbs_bf"] = (dt * np.asarray(inputs["out_b"], f32)[None, :]).astype(bf16)

    means = np.asarray(inputs["target_means"], f32)
    row4 = np.array([[0.5 * LOGN, EPS_A * LOGN, -0.05 * dt * LOGN,
                      0.1 * dt * LOGN]], np.float64)
    cp = np.zeros((128, 97), f32)
    cp[0:D, 0:M] = means.T
    cp[0:M, 8:8 + D] = means
    cp[0:1, 72:72 + M] = (-0.5 * (means.astype(np.float64) ** 2).sum(-1)
                          )[None, :]
    cp[0:M, 80:80 + NB] = np.tile((-dt * betas[:NB])[None, :], (M, 1))
    cp[0:1, 88:92] = row4

    # host bandwidth for steps 0 and 1: hL = h*logn from x0 subsample
    f = x0[:128, None, :] - x0[None, :, :]
    z = (f * f).sum(-1) + EPS_A
    m, m2 = z.mean(), (z * z).mean()
    q = (m2 - m * m) / (8.0 * m * m)
    P = m * (1.0 - q) ** 2
    hL0 = P - EPS_A + EPS_A * EPS_A / (4.0 * P)
    cp[0:1, 92:96] = row4 / hL0
    cp[:, 96:97] = 1.0 - dt
    g["cpack"] = cp

    # full-particle tiles for step 0
    x0f = x0.astype(f32)
    g["xall0"] = np.ascontiguousarray(-2.0 * x0f.T).astype(bf16)
    x2 = (x0f * x0f).sum(-1) + 1.0
    g["x2rowN2_0"] = (-2.0 * x2[None, :]).astype(bf16)

    # noise, pre-scaled, [B, NB, D]
    noi = np.asarray(inputs["noises"], f32) * np.float32(np.sqrt(2.0 * dt))
    g["noises_all"] = np.ascontiguousarray(noi.transpose(1, 0, 2))
    g["x0f"] = x0f
    return g


def _shard(g, c):
    import ml_dtypes
    bf16 = ml_dtypes.bfloat16
    sl = slice(c * BL, (c + 1) * BL)
    m = {k: g[k] for k in ["inWs_bf", "te_bf", "hW_bf", "hb_bf", "outWs_bf",
                           "outbs_bf", "cpack", "xall0", "x2rowN2_0"]}
    x0l = np.ascontiguousarray(g["x0f"][sl])
    m["x0_loc"] = x0l
    m["xT0_loc"] = np.ascontiguousarray(x0l.T)
    m["xT0_locN2"] = np.ascontiguousarray(-2.0 * x0l.T).astype(bf16)
    x2 = (x0l * x0l).sum(-1) + 1.0
    m["x2locn2_0"] = (-2.0 * x2[None, :]).astype(bf16)
    m["noises_s"] = np.ascontiguousarray(g["noises_all"][sl])
    return m


def run(inputs, trace=False, trace_cores=None):
    from concourse.bass_utils import run_bass_kernel_spmd
    nc = _get_nc()
    g = _prep(inputs)
    in_maps = [_shard(g, c) for c in range(NCORES)]
    res = run_bass_kernel_spmd(nc, in_maps, core_ids=list(range(NCORES)),
                               trace=trace, trace_cores=trace_cores)
    out = np.zeros((NB + 1, B, D), np.float32)
    out[0] = np.asarray(inputs["particles"], np.float32)
    for c in range(NCORES):
        out[1:, c * BL:(c + 1) * BL, :] = \
            np.asarray(res.results[c]["traj"]).reshape(NB, BL, D)
    return out, res


def kernel(**inputs):
    return run(inputs)[0]
